# revision 1
# baseline (speedup 1.0000x reference)
"""Expert-parallel Trainium2 kernel for PlasticityModelMoE.

Sharding: core c owns expert c (expert_w/b, conn_*). Each core computes its
expert's gate-weighted contribution y_c = gate[:,c] * relu((x@W_c + b_c) * m_c)
for the full batch, then per-512-column ReduceScatter(add) chunks leave core c
with batch rows [128c, 128c+128) of moe_out. Stage 2 (episodic-memory
attention + blended learned activation) runs batch-parallel on those 128 rows
with replicated mem_read_w / memory. Host concatenates the 8 [128, 2048]
outputs.

DMA rings: sync(SP) carries inputs + y writes + mem; ACT ring prefetches
mem_read_w from t=0; gpsimd ring interleaves RS chunks with moe loads so the
collective never blocks weight prefetch.
"""

import numpy as np

B, D, H, E, M = 1024, 1024, 2048, 8, 2048
NCORES = 8
SELU_SCALE = 1.0507009873554805
SELU_ALPHA = 1.6732632423543772

_CACHED_NC = {}
_LAST_H1 = H
_LAST_IN_MAPS = None


def _build_program(h1):
    import concourse.bass as bass
    from concourse import bacc, mybir, tile
    from concourse.masks import make_identity

    f32 = mybir.dt.float32
    f32r = mybir.dt.float32r
    rr = lambda ap: ap.bitcast(f32r)
    NH = h1 // 512   # stage-1 column chunks
    KH = h1 // 128   # stage-1 K blocks for the attention logits
    AF = mybir.ActivationFunctionType
    ALU = mybir.AluOpType
    AX = mybir.AxisListType

    nc = bacc.Bacc(None, target_bir_lowering=False, debug=False)

    xT_d = nc.dram_tensor("xT", [D, B], f32, kind="ExternalInput")
    gw_d = nc.dram_tensor("gw", [128, 8, E], f32, kind="ExternalInput")
    ew_d = nc.dram_tensor("ew", [D, H], f32, kind="ExternalInput")
    eb_d = nc.dram_tensor("eb", [1, H], f32, kind="ExternalInput")
    cw1_d = nc.dram_tensor("cw1", [H, 32], f32, kind="ExternalInput")
    cb1_d = nc.dram_tensor("cb1", [32, 1], f32, kind="ExternalInput")
    cw2_d = nc.dram_tensor("cw2", [32, H], f32, kind="ExternalInput")
    cb2_d = nc.dram_tensor("cb2", [1, H], f32, kind="ExternalInput")
    na_d = nc.dram_tensor("na", [128, 16], f32, kind="ExternalInput")
    nm_d = nc.dram_tensor("nm", [1, H], f32, kind="ExternalInput")
    mrw_d = nc.dram_tensor("mrw", [H, M], f32, kind="ExternalInput")
    mrb_d = nc.dram_tensor("mrb", [1, M], f32, kind="ExternalInput")
    mem_d = nc.dram_tensor("mem", [M, H], f32, kind="ExternalInput")
    cf_d = nc.dram_tensor("coef", [1, 8], f32, kind="ExternalInput")
    out_d = nc.dram_tensor("out", [128, H], f32, kind="ExternalOutput")

    dma = nc.default_dma_engine   # SP hwdge ring
    adma = nc.scalar              # Activation hwdge ring (2nd DGE)
    gdma = nc.gpsimd              # gpsimd ring (shared with collectives)

    with tile.TileContext(nc) as tc:
        with tc.tile_pool(name="consts", bufs=1) as consts, \
             tc.tile_pool(name="dramp", bufs=1, space="DRAM") as dramp, \
             tc.tile_pool(name="mrwp", bufs=min(KH, 8)) as mrwp:

            identity = consts.tile([128, 128], f32, tag="idn")
            make_identity(nc, identity)
            ones_row = consts.tile([1, 128], f32, tag="ones")
            nc.vector.memset(ones_row, 1.0)
            ones_r = consts.tile([1, 128], f32r, tag="onesr")
            nc.scalar.copy(ones_r, ones_row)
            coef_row = consts.tile([1, 8], f32, tag="coef")
            dma.dma_start(coef_row, cf_d[:])
            coeffs_bc = consts.tile([128, 8], f32, tag="cfb")
            moe_sb = consts.tile([128, h1], f32, tag="moe")

            ys = [dramp.tile([B, 512], f32, tag=f"y{n}", name=f"y{n}")
                  for n in range(NH)]
            rss = [dramp.tile([128, 512], f32, tag=f"rs{n}", name=f"rs{n}")
                   for n in range(NH)]

            # prefetch mem_read_w rows on the ACT DGE ring from t=0
            mrw_tiles = []
            for hk in range(KH):
                t_ = mrwp.tile([128, M], f32r, tag="w", name=f"mrw{hk}")
                adma.dma_start(t_, rr(mrw_d[hk * 128:(hk + 1) * 128, :]))
                mrw_tiles.append(t_)

            # ---------------- stage 1: expert-parallel MoE ----------------
            with tc.tile_pool(name="w1", bufs=1) as w1:
                xT_sb = w1.tile([128, 8, B], f32, tag="xT")
                for k in range(8):
                    dma.dma_start(rr(xT_sb[:, k, :]),
                                  rr(xT_d[k * 128:(k + 1) * 128, :]))
                gw_sb = w1.tile([128, 8, E], f32, tag="gw")
                dma.dma_start(gw_sb, gw_d[:])
                ew_sb = w1.tile([128, 8, h1], f32r, tag="ew")
                eb_row = w1.tile([1, h1], f32r, tag="eb")
                dma.dma_start(eb_row, rr(eb_d[0:1, 0:h1]))
                cmask_bc = w1.tile([128, h1], f32, tag="cmb")

                # conn MLP (per expert, from neuron_avg) -> cmask = sigmoid(.)*mask
                with tc.tile_pool(name="cw", bufs=1) as cw, \
                     tc.tile_pool(name="pconn", bufs=1, space="PSUM") as pc:
                    cw1_sb = cw.tile([128, 16, 32], f32, tag="cw1")
                    for k in range(16):
                        dma.dma_start(cw1_sb[:, k, :], cw1_d[k * 128:(k + 1) * 128, :])
                    na_sb = cw.tile([128, 16], f32, tag="na")
                    dma.dma_start(na_sb, na_d[:])
                    cb1_col = cw.tile([32, 1], f32, tag="cb1")
                    dma.dma_start(cb1_col, cb1_d[:])
                    cw2_sb = cw.tile([32, h1], f32, tag="cw2")
                    dma.dma_start(cw2_sb, cw2_d[:, 0:h1])
                    cb2_row = cw.tile([1, h1], f32, tag="cb2")
                    dma.dma_start(cb2_row, cb2_d[0:1, 0:h1])
                    nm_row = cw.tile([1, h1], f32, tag="nm")
                    dma.dma_start(nm_row, nm_d[0:1, 0:h1])

                    # expert weights, chunk-major so chunk 0 lands first
                    for n in range(NH):
                        for k in range(8):
                            dma.dma_start(
                                ew_sb[:, k, n * 512:(n + 1) * 512],
                                rr(ew_d[k * 128:(k + 1) * 128,
                                        n * 512:(n + 1) * 512]))

                    h1_ps = pc.tile([32, 1], f32, tag="h1")
                    for k in range(16):
                        nc.tensor.matmul(h1_ps, cw1_sb[:, k, :], na_sb[:, k:k + 1],
                                         start=(k == 0), stop=(k == 15))
                    h1_sb = cw.tile([32, 1], f32, tag="h1s")
                    nc.scalar.activation(h1_sb, h1_ps, AF.Relu, bias=cb1_col)

                    conn_ps = pc.tile([1, h1], f32, tag="conn")
                    for n in range(NH):
                        sl = slice(n * 512, (n + 1) * 512)
                        nc.tensor.matmul(conn_ps[0:1, sl], h1_sb, cw2_sb[:, sl],
                                         start=True, stop=False)
                        nc.tensor.matmul(conn_ps[0:1, sl], ones_row[0:1, 0:1],
                                         cb2_row[0:1, sl], start=False, stop=True)
                    conn_row = cw.tile([1, h1], f32, tag="cr")
                    nc.scalar.activation(conn_row, conn_ps[0:1, :], AF.Sigmoid)
                    cmask_row = cw.tile([1, h1], f32, tag="cmr")
                    nc.vector.tensor_tensor(cmask_row, conn_row, nm_row, ALU.mult)

                    for n in range(NH):
                        sl = slice(n * 512, (n + 1) * 512)
                        bc_ps = pc.tile([128, 512], f32, tag="bc", bufs=2, name=f"bc{n}")
                        nc.tensor.matmul(bc_ps, ones_row, cmask_row[0:1, sl],
                                         start=True, stop=True)
                        nc.scalar.copy(cmask_bc[:, sl], bc_ps)

                    cf_ps = pc.tile([128, 8], f32, tag="cf")
                    nc.tensor.matmul(cf_ps, ones_row, coef_row, start=True, stop=True)
                    nc.scalar.copy(coeffs_bc, cf_ps)

                # gate softmax for all batch blocks, then chunk-major z compute
                # with a ReduceScatter issued as soon as each chunk is written
                with tc.tile_pool(name="bl", bufs=1) as bl, \
                     tc.tile_pool(name="pb", bufs=1, space="PSUM") as pb:
                    gcols = []
                    for i in range(8):
                        bs = slice(i * 128, (i + 1) * 128)
                        gate_ps = pb.tile([128, E], f32, tag="g", bufs=2, name=f"g{i}")
                        for k in range(8):
                            nc.tensor.matmul(gate_ps, xT_sb[:, k, bs], gw_sb[:, k, :],
                                             start=(k == 0), stop=(k == 7))
                        ngm = bl.tile([128, 1], f32, tag="ngm", bufs=2, name=f"ngm{i}")
                        nc.vector.reduce_max(ngm, gate_ps, axis=AX.X, negate=True)
                        eg = bl.tile([128, E], f32, tag="eg", bufs=2, name=f"eg{i}")
                        sume = bl.tile([128, 1], f32, tag="se", bufs=2, name=f"se{i}")
                        nc.scalar.activation(eg, gate_ps, AF.Exp, bias=ngm,
                                             accum_out=sume)
                        rec = bl.tile([128, 1], f32, tag="rec", bufs=2, name=f"rec{i}")
                        nc.vector.reciprocal(rec, sume)
                        gcol = bl.tile([128, 1], f32, tag=f"gc{i}", name=f"gc{i}")
                        nc.vector.tensor_scalar_mul(gcol, eg[:, 0:1], rec)
                        gcols.append(gcol)

                    for n in range(NH):
                        sl = slice(n * 512, (n + 1) * 512)
                        for i in range(8):
                            bs = slice(i * 128, (i + 1) * 128)
                            z_ps = pb.tile([128, 512], f32, tag="z", bufs=4,
                                           name=f"z{n}_{i}")
                            for k in range(8):
                                nc.tensor.matmul(z_ps, rr(xT_sb[:, k, bs]),
                                                 ew_sb[:, k, sl],
                                                 start=(k == 0), stop=False)
                            nc.tensor.matmul(z_ps, ones_r, eb_row[0:1, sl],
                                             start=False, stop=True)
                            r_sb = bl.tile([128, 512], f32, tag="r", bufs=3,
                                           name=f"r{n}_{i}")
                            nc.scalar.activation(r_sb, z_ps, AF.Relu, scale=gcols[i])
                            y_sb = bl.tile([128, 512], f32, tag="yc", bufs=3,
                                           name=f"yc{n}_{i}")
                            nc.vector.tensor_tensor(y_sb, r_sb,
                                                    cmask_bc[:, sl], ALU.mult)
                            dma.dma_start(ys[n][bs, :], y_sb)
                        nc.gpsimd.collective_compute(
                            "ReduceScatter",
                            bass.mybir.AluOpType.add,
                            replica_groups=[[0, 1, 2, 3, 4, 5, 6, 7]],
                            ins=[ys[n].opt()],
                            outs=[rss[n].opt()],
                        )
                        gdma.dma_start(moe_sb[:, sl], rss[n])

            # ---------------- stage 2: memory read + learned activation ------
            with tc.tile_pool(name="st2", bufs=1) as st2:
                mrb_row = st2.tile([1, M], f32r, tag="mrb")
                dma.dma_start(mrb_row, rr(mrb_d[:]))
                moeT_sb = st2.tile([128, KH * 128], f32r, tag="moeT")
                exp_sb = st2.tile([128, M], f32, tag="exp")
                expT_sb = st2.tile([128, 16 * 128], f32r, tag="expT")
                s_sb = st2.tile([128, H], f32, tag="s")
                out_sb = st2.tile([128, H], f32, tag="o")
                srec = st2.tile([128, 1], f32, tag="srec")

                with tc.tile_pool(name="pt", bufs=1, space="PSUM") as pt:
                    with tc.tile_pool(name="plg", bufs=1, space="PSUM") as plg:
                        lg = [plg.tile([128, 512], f32, tag="lg", bufs=4,
                                       name=f"lg{n}") for n in range(4)]
                        for ch in range(NH):
                            tp = pt.tile([128, 512], f32, tag="tp", bufs=2,
                                         name=f"tpm{ch}")
                            for j in range(4):
                                hk = ch * 4 + j
                                nc.tensor.transpose(tp[:, j * 128:(j + 1) * 128],
                                                    moe_sb[:, hk * 128:(hk + 1) * 128],
                                                    identity)
                            nc.scalar.copy(moeT_sb[:, ch * 512:(ch + 1) * 512], tp)
                            for j in range(4):
                                hk = ch * 4 + j
                                for n in range(4):
                                    nc.tensor.matmul(
                                        lg[n],
                                        moeT_sb[:, hk * 128:(hk + 1) * 128],
                                        mrw_tiles[hk][:, n * 512:(n + 1) * 512],
                                        start=(hk == 0), stop=False)
                        for n in range(4):
                            nc.tensor.matmul(lg[n], ones_r,
                                             mrb_row[0:1, n * 512:(n + 1) * 512],
                                             start=False, stop=True)

                        nmx = []
                        for n in range(4):
                            t_ = st2.tile([128, 1], f32, tag=f"nmx{n}", name=f"nmx{n}")
                            nc.vector.reduce_max(t_, lg[n], axis=AX.X, negate=True)
                            nmx.append(t_)
                        t01 = st2.tile([128, 1], f32, tag="t01")
                        nc.vector.tensor_scalar_min(t01, nmx[0], nmx[1])
                        t23 = st2.tile([128, 1], f32, tag="t23")
                        nc.vector.tensor_scalar_min(t23, nmx[2], nmx[3])
                        ngm2 = st2.tile([128, 1], f32, tag="ngm2")
                        nc.vector.tensor_scalar_min(ngm2, t01, t23)
                        ses = []
                        for n in range(4):
                            se_ = st2.tile([128, 1], f32, tag=f"ses{n}", name=f"ses{n}")
                            nc.scalar.activation(exp_sb[:, n * 512:(n + 1) * 512],
                                                 lg[n], AF.Exp, bias=ngm2,
                                                 accum_out=se_)
                            ses.append(se_)
                        s01 = st2.tile([128, 1], f32, tag="s01")
                        nc.vector.tensor_tensor(s01, ses[0], ses[1], ALU.add)
                        s23 = st2.tile([128, 1], f32, tag="s23")
                        nc.vector.tensor_tensor(s23, ses[2], ses[3], ALU.add)
                        stot = st2.tile([128, 1], f32, tag="stot")
                        nc.vector.tensor_tensor(stot, s01, s23, ALU.add)
                        nc.vector.reciprocal(srec, stot)

                    for t in range(4):
                        tp = pt.tile([128, 512], f32, tag="tp", bufs=2, name=f"tpe{t}")
                        for j in range(4):
                            mk = t * 4 + j
                            nc.tensor.transpose(tp[:, j * 128:(j + 1) * 128],
                                                exp_sb[:, mk * 128:(mk + 1) * 128],
                                                identity)
                        nc.scalar.copy(expT_sb[:, t * 512:(t + 1) * 512], tp)

                with tc.tile_pool(name="memp", bufs=10) as memp, \
                     tc.tile_pool(name="prd", bufs=1, space="PSUM") as prd:
                    rd = [prd.tile([128, 512], f32, tag="rd", bufs=4,
                                   name=f"rd{n}") for n in range(4)]
                    for mk in range(16):
                        mem_t = memp.tile([128, H], f32r, tag="m", name=f"mem{mk}")
                        dma.dma_start(mem_t, rr(mem_d[mk * 128:(mk + 1) * 128, :]))
                        for n in range(4):
                            nc.tensor.matmul(rd[n],
                                             expT_sb[:, mk * 128:(mk + 1) * 128],
                                             mem_t[:, n * 512:(n + 1) * 512],
                                             start=(mk == 0), stop=(mk == 15))
                    # s = moe + read_vec/sum  (deferred softmax normalization);
                    # columns >= h1 have moe == 0 by mask structure
                    for n in range(4):
                        sl = slice(n * 512, (n + 1) * 512)
                        if n < NH:
                            nc.vector.scalar_tensor_tensor(s_sb[:, sl], rd[n],
                                                           srec, moe_sb[:, sl],
                                                           ALU.mult, ALU.add)
                        else:
                            nc.vector.tensor_scalar_mul(s_sb[:, sl], rd[n], srec)

                # blended learned activation via diag-matmul accumulation.
                # Mish is not in any HW act table; synthesize
                # mish(s) = s * tanh(relu(s) + ln(1 + exp(-|s|))).
                with tc.tile_pool(name="pac", bufs=1, space="PSUM") as pac, \
                     tc.tile_pool(name="brp", bufs=1) as brp:
                    acc = [pac.tile([128, 512], f32, tag="acc", bufs=4,
                                    name=f"acc{n}") for n in range(4)]
                    n_groups = 7
                    gi = [0]

                    def acc_branch(br_tile, ci):
                        diag = brp.tile([128, 128], f32r, tag="d", bufs=2,
                                        name=f"d{gi[0]}")
                        nc.vector.tensor_scalar_mul(diag, identity,
                                                    coeffs_bc[:, ci:ci + 1])
                        for n in range(4):
                            nc.tensor.matmul(acc[n], diag,
                                             br_tile[:, n * 512:(n + 1) * 512],
                                             start=(gi[0] == 0),
                                             stop=(gi[0] == n_groups - 1))
                        gi[0] += 1

                    # --- nl_exp table phase ---
                    relu_br = brp.tile([128, H], f32r, tag="relu")
                    nc.scalar.activation(relu_br, s_sb, AF.Relu)
                    acc_branch(relu_br, 5)
                    # exp(min(s,0)) branch; the -1 of expm1 is folded into the
                    # final subtraction of c_em below
                    mn = brp.tile([128, H], f32, tag="sc1", bufs=2, name="mn")
                    nc.vector.tensor_scalar_min(mn, s_sb, 0.0)
                    em_br = brp.tile([128, H], f32r, tag="b", bufs=2, name="em")
                    nc.scalar.activation(em_br, mn, AF.Exp)
                    acc_branch(em_br, 6)
                    # softplus(s) = relu(s) + ln(1 + exp(-|s|))
                    abs_s = brp.tile([128, H], f32, tag="sc2", bufs=2, name="ab")
                    nc.scalar.activation(abs_s, s_sb, AF.Abs)
                    enab = brp.tile([128, H], f32, tag="sc1", bufs=2, name="en")
                    nc.scalar.activation(enab, abs_s, AF.Exp, scale=-1.0)
                    ep1 = brp.tile([128, H], f32, tag="sc2", bufs=2, name="e1")
                    nc.vector.tensor_scalar_add(ep1, enab, 1.0)
                    ln1p = brp.tile([128, H], f32, tag="sc1", bufs=2, name="ln")
                    nc.scalar.activation(ln1p, ep1, AF.Ln)
                    sp = brp.tile([128, H], f32, tag="sp")
                    nc.vector.tensor_tensor(sp, ln1p, relu_br.bitcast(f32),
                                            ALU.add)
                    # --- sigmoid table phase (Sigmoid + Tanh) ---
                    sg_br = brp.tile([128, H], f32r, tag="b", bufs=2, name="sg")
                    nc.scalar.activation(sg_br, s_sb, AF.Sigmoid)
                    acc_branch(sg_br, 0)
                    th_br = brp.tile([128, H], f32r, tag="b", bufs=2, name="th")
                    nc.scalar.activation(th_br, s_sb, AF.Tanh)
                    acc_branch(th_br, 1)
                    mt = brp.tile([128, H], f32, tag="sc2", bufs=2, name="mt")
                    nc.scalar.activation(mt, sp, AF.Tanh)
                    mish_br = brp.tile([128, H], f32r, tag="b", bufs=2, name="mi")
                    nc.vector.tensor_tensor(mish_br, mt, s_sb, ALU.mult)
                    acc_branch(mish_br, 4)
                    # --- silu table phase ---
                    sl_br = brp.tile([128, H], f32r, tag="b", bufs=2, name="sl")
                    nc.scalar.activation(sl_br, s_sb, AF.Silu)
                    acc_branch(sl_br, 2)
                    # --- gelu table phase ---
                    gl_br = brp.tile([128, H], f32r, tag="b", bufs=2, name="gl")
                    nc.scalar.activation(gl_br, s_sb, AF.Gelu)
                    acc_branch(gl_br, 3)
                    assert gi[0] == n_groups
                    for n in range(4):
                        nc.vector.tensor_scalar_sub(out_sb[:, n * 512:(n + 1) * 512],
                                                    acc[n], coeffs_bc[:, 6:7])
                dma.dma_start(out_d[:, :], out_sb)
    nc.finalize()
    return nc


def _get_nc(h1=None):
    if h1 is None:
        h1 = _LAST_H1
    if h1 not in _CACHED_NC:
        _CACHED_NC[h1] = _build_program(h1)
    return _CACHED_NC[h1]


def _r12(a):
    """Round fp32 to the fp32r grid (11 explicit mantissa bits, RNE)."""
    u = np.ascontiguousarray(a).view(np.uint32)
    u = (u + np.uint32(0x7FF) + ((u >> np.uint32(12)) & np.uint32(1))) \
        & np.uint32(0xFFFFF000)
    return u.view(np.float32)


def kernel(**inputs):
    from concourse.bass_utils import run_bass_kernel_spmd

    f = lambda a: np.ascontiguousarray(np.asarray(a, dtype=np.float32))
    x = f(inputs["x"])
    gate_w = f(inputs["gate_w"])
    expert_w = f(inputs["expert_w"])
    expert_b = f(inputs["expert_b"])
    conn_w1 = f(inputs["conn_w1"])
    conn_b1 = f(inputs["conn_b1"])
    conn_w2 = f(inputs["conn_w2"])
    conn_b2 = f(inputs["conn_b2"])
    neuron_avg = f(inputs["neuron_avg"])
    neuron_mask = f(inputs["neuron_mask"])
    mem_read_w = f(inputs["mem_read_w"])
    mem_read_b = f(inputs["mem_read_b"])
    memory = f(inputs["memory"])
    act_w = f(inputs["act_w"]).reshape(-1)

    # host prep: softmax blend weights -> 7 branch coefficients
    p = np.exp(act_w - act_w.max())
    p = p / p.sum()
    coef = np.array([[p[0], p[2], p[4], p[5], p[7],
                      p[1] + p[3] + p[6] * SELU_SCALE,
                      p[1] + p[6] * SELU_SCALE * SELU_ALPHA, 0.0]], np.float32)

    # operands of float32r matmuls must carry fp32r-rounded bits
    xT = _r12(np.ascontiguousarray(x.T))
    expert_w = _r12(expert_w)
    expert_b = _r12(expert_b)
    mem_read_w = _r12(mem_read_w)
    memory = _r12(memory)
    mrb = _r12(np.ascontiguousarray(mem_read_b.reshape(1, M)))

    in_maps = []
    for c in range(NCORES):
        gwr = np.roll(gate_w, -c, axis=1)  # own expert -> column 0
        in_maps.append({
            "xT": xT,
            "gw": np.ascontiguousarray(gwr.reshape(8, 128, E).transpose(1, 0, 2)),
            "ew": np.ascontiguousarray(expert_w[c]),
            "eb": np.ascontiguousarray(expert_b[c].reshape(1, H)),
            "cw1": np.ascontiguousarray(conn_w1[c]),
            "cb1": np.ascontiguousarray(conn_b1[c].reshape(32, 1)),
            "cw2": np.ascontiguousarray(conn_w2[c]),
            "cb2": np.ascontiguousarray(conn_b2[c].reshape(1, H)),
            "na": np.ascontiguousarray(neuron_avg[c].reshape(16, 128).T),
            "nm": np.ascontiguousarray(neuron_mask[c].reshape(1, H)),
            "mrw": mem_read_w,
            "mrb": mrb,
            "mem": memory,
            "coef": coef,
        })

    # stage-1 live width: columns past the last nonzero mask column are
    # structurally zero in moe_out, so the program skips them entirely
    nz = np.nonzero(neuron_mask.any(axis=0))[0]
    h1 = int(nz[-1]) + 1 if nz.size else 512
    h1 = min(H, max(512, -(-h1 // 512) * 512))

    global _LAST_IN_MAPS, _LAST_H1
    _LAST_IN_MAPS = in_maps
    _LAST_H1 = h1
    nc = _get_nc(h1)
    results = run_bass_kernel_spmd(nc, in_maps, list(range(NCORES))).results
    out = np.concatenate(
        [np.asarray(results[c]["out"], dtype=np.float32) for c in range(NCORES)],
        axis=0)
    return out



# revision 4
# speedup vs baseline: 1.6589x; 1.6589x over previous
"""Hybrid expert/batch-parallel Trainium2 kernel for PlasticityModelMoE.

Sharding (g=2 expert-parallel x b=4 batch-parallel): core c = (batch group
bg=c>>1, expert group eg=c&1). Each core computes its 4 experts' gate-weighted
sum for its 256 batch rows, then ONE pairwise ReduceScatter(add) over
[[0,1],[2,3],[4,5],[6,7]] (512KB) leaves core c with batch rows
[128c, 128c+128). Stage 2 (episodic-memory attention + blended learned
activation) runs on those 128 rows with replicated mem_read_w / memory in
bf16. Host concatenates the 8 [128, 2048] outputs.

Host-side folding: the DynamicConnectivity MLP depends only on neuron_avg
(batch-independent), so cmask = sigmoid(conn)*neuron_mask is computed on the
host and folded into expert_w columns (relu(z*m) == m*relu(z) for m>=0);
device stage 1 is then just gate softmax + z matmuls + relu/gate-scale
accumulation. Columns past the last nonzero mask column are structurally zero
in moe_out, so only h1 columns are computed (and only h1 rows of mem_read_w
are loaded).

DMA rings: sync + vector stream the folded expert weights (half-expert
tiles); scalar(ACT) prefetches mem_read_w then memory (bf16); gpsimd carries
xT/gw/coef, the RS traffic, and the output.
"""

import numpy as np

B, D, H, E, M = 1024, 1024, 2048, 8, 2048
NCORES = 8
SELU_SCALE = 1.0507009873554805
SELU_ALPHA = 1.6732632423543772

_CACHED_NC = {}
_LAST_KEY = None
_LAST_IN_MAPS = None


def _build_program(h1, g, has_eb, has_mrb):
    import concourse.bass as bass
    from concourse import bacc, mybir, tile
    from concourse.masks import make_identity

    f32 = mybir.dt.float32
    f32r = mybir.dt.float32r
    f16 = mybir.dt.float16
    EL = E // g          # experts per core
    NB = g               # 128-row batch blocks per core
    NH = h1 // 512       # stage-1 column chunks
    KH = h1 // 128       # K blocks for the attention logits
    AF = mybir.ActivationFunctionType
    ALU = mybir.AluOpType
    AX = mybir.AxisListType

    nc = bacc.Bacc(None, target_bir_lowering=False, debug=False)

    xT_d = nc.dram_tensor("xT", [128, 8, NB * 128], f16, kind="ExternalInput")
    gw_d = nc.dram_tensor("gw", [128, 8, E], f16, kind="ExternalInput")
    ew_d = nc.dram_tensor("ew", [EL, 128, 8, h1], f16, kind="ExternalInput")
    if has_eb:
        eb_d = nc.dram_tensor("eb", [EL, 1, h1], f16, kind="ExternalInput")
    mrw_d = nc.dram_tensor("mrw", [KH, 128, M], f16, kind="ExternalInput")
    if has_mrb:
        mrb_d = nc.dram_tensor("mrb", [1, M], f32, kind="ExternalInput")
    mem_d = nc.dram_tensor("mem", [16, 128, H], f16, kind="ExternalInput")
    cf_d = nc.dram_tensor("coef", [1, 8], f32, kind="ExternalInput")
    out_d = nc.dram_tensor("out", [128, H], f32, kind="ExternalOutput")

    dma = nc.default_dma_engine   # SP hwdge ring: ew h0 + mem[0:8]
    adma = nc.scalar              # ACT ring: ew h1 + mrw + mem[8:16]
    gdma = nc.gpsimd              # gpsimd ring: xT/gw/coef + RS + out

    with tile.TileContext(nc) as tc:
        with tc.tile_pool(name="consts", bufs=1) as consts, \
             tc.tile_pool(name="dramp", bufs=1, space="DRAM") as dramp, \
             tc.tile_pool(name="mrwp", bufs=KH) as mrwp, \
             tc.tile_pool(name="memp", bufs=16) as memp:

            identity = consts.tile([128, 128], f32, tag="idn")
            make_identity(nc, identity)
            ones_row = consts.tile([1, 128], f32, tag="ones")
            nc.vector.memset(ones_row, 1.0)
            coef_row = consts.tile([1, 8], f32, tag="coef")
            gdma.dma_start(coef_row, cf_d[:])
            coeffs_bc = consts.tile([128, 8], f32, tag="cfb")
            moe_r = consts.tile([128, h1], f32, tag="moer")


            if g > 1:
                ys = [dramp.tile([NB * 128, 512], f32, tag=f"y{n}", name=f"y{n}")
                      for n in range(NH)]
                rss = [dramp.tile([128, 512], f32, tag=f"rs{n}", name=f"rs{n}")
                       for n in range(NH)]
                groups = [[2 * k, 2 * k + 1] for k in range(4)]

            # ---------------- stage 1: hybrid-parallel MoE ----------------
            with tc.tile_pool(name="w1", bufs=1) as w1, \
                 tc.tile_pool(name="ewp", bufs=4) as ewp, \
                 tc.tile_pool(name="pb", bufs=1, space="PSUM") as pb:
                xT_sb = w1.tile([128, 8, NB * 128], f16, tag="xT")
                gdma.dma_start(xT_sb, xT_d[:])
                gw_sb = w1.tile([128, 8, E], f16, tag="gw")
                gdma.dma_start(gw_sb, gw_d[:])
                if has_eb:
                    eb_rows = w1.tile([EL, 1, h1], f16, tag="eb")
                    dma.dma_start(eb_rows, eb_d[:])
                    ones_f16 = w1.tile([1, 128], f16, tag="o16")
                    nc.vector.memset(ones_f16, 1.0)

                # expert weight stream: half-expert tiles on two rings
                ew_tiles = []
                for e in range(EL):
                    t0 = ewp.tile([128, 4, h1], f16, tag="ew", name=f"ew{e}h0")
                    dma.dma_start(t0, ew_d[e, :, 0:4, :])
                    t1 = ewp.tile([128, 4, h1], f16, tag="ew", name=f"ew{e}h1")
                    adma.dma_start(t1, ew_d[e, :, 4:8, :])
                    ew_tiles.append((t0, t1))

                # stage-2 weights queued behind the expert stream
                mrw_tiles = []
                for hk in range(KH):
                    t_ = mrwp.tile([128, M], f16, tag="w", name=f"mrw{hk}")
                    adma.dma_start(t_, mrw_d[hk])
                    mrw_tiles.append(t_)
                mem_tiles = []
                for mk in range(16):
                    t_ = memp.tile([128, H], f16, tag="m", name=f"mem{mk}")
                    (dma if mk < 8 else adma).dma_start(t_, mem_d[mk])
                    mem_tiles.append(t_)

                # broadcast activation-blend coefficients to 128 partitions
                cf_ps = pb.tile([128, 8], f32, tag="cf")
                nc.tensor.matmul(cf_ps, ones_row, coef_row, start=True, stop=True)
                nc.scalar.copy(coeffs_bc, cf_ps)

                # gate softmax per batch block -> per-own-expert columns
                gcols = []
                for i in range(NB):
                    bs = slice(i * 128, (i + 1) * 128)
                    gate_ps = pb.tile([128, E], f32, tag="g", bufs=2, name=f"g{i}")
                    for k in range(8):
                        nc.tensor.matmul(gate_ps, xT_sb[:, k, bs], gw_sb[:, k, :],
                                         start=(k == 0), stop=(k == 7))
                    ngm = w1.tile([128, 1], f32, tag="ngm", bufs=2, name=f"ngm{i}")
                    nc.vector.reduce_max(ngm, gate_ps, axis=AX.X, negate=True)
                    eg_t = w1.tile([128, E], f32, tag="eg", bufs=2, name=f"eg{i}")
                    sume = w1.tile([128, 1], f32, tag="se", bufs=2, name=f"se{i}")
                    nc.scalar.activation(eg_t, gate_ps, AF.Exp, bias=ngm,
                                         accum_out=sume)
                    rec = w1.tile([128, 1], f32, tag="rec", bufs=2, name=f"rec{i}")
                    nc.vector.reciprocal(rec, sume)
                    cols = []
                    for j in range(EL):
                        gcol = w1.tile([128, 1], f32, tag=f"gc{i}_{j}",
                                       name=f"gc{i}_{j}")
                        nc.vector.tensor_scalar_mul(gcol, eg_t[:, j:j + 1], rec)
                        cols.append(gcol)
                    gcols.append(cols)

                moe_parts = [moe_r] if g == 1 else [
                    w1.tile([128, h1], f32, tag=f"mp{i}", name=f"mp{i}")
                    for i in range(NB)]

                for e in range(EL):
                    t0, t1 = ew_tiles[e]
                    for n in range(NH):
                        sl = slice(n * 512, (n + 1) * 512)
                        for i in range(NB):
                            bs = slice(i * 128, (i + 1) * 128)
                            z_ps = pb.tile([128, 512], f32, tag="z", bufs=4,
                                           name=f"z{e}_{n}_{i}")
                            for k in range(4):
                                nc.tensor.matmul(z_ps, xT_sb[:, k, bs],
                                                 t0[:, k, sl],
                                                 start=(k == 0), stop=False)
                            for k in range(4):
                                last = (k == 3) and not has_eb
                                nc.tensor.matmul(z_ps, xT_sb[:, 4 + k, bs],
                                                 t1[:, k, sl],
                                                 start=False, stop=last)
                            if has_eb:
                                nc.tensor.matmul(z_ps, ones_f16[0:1, 0:1],
                                                 eb_rows[e, 0:1, sl],
                                                 start=False, stop=True)
                            if e == 0:
                                nc.scalar.activation(moe_parts[i][:, sl], z_ps,
                                                     AF.Relu, scale=gcols[i][e])
                            else:
                                t_ = w1.tile([128, 512], f32, tag="acc", bufs=3,
                                             name=f"a{e}_{n}_{i}")
                                nc.scalar.activation(t_, z_ps, AF.Relu,
                                                     scale=gcols[i][e])
                                nc.vector.tensor_tensor(moe_parts[i][:, sl],
                                                        moe_parts[i][:, sl],
                                                        t_, ALU.add)

                # pairwise ReduceScatter per column chunk
                if g > 1:
                    for n in range(NH):
                        sl = slice(n * 512, (n + 1) * 512)
                        for i in range(NB):
                            gdma.dma_start(ys[n][i * 128:(i + 1) * 128, :],
                                           moe_parts[i][:, sl])
                        nc.gpsimd.collective_compute(
                            "ReduceScatter",
                            bass.mybir.AluOpType.add,
                            replica_groups=groups,
                            ins=[ys[n].opt()],
                            outs=[rss[n].opt()],
                        )
                        gdma.dma_start(moe_r[:, sl], rss[n])

            # ---------------- stage 2: memory read + learned activation ------
            with tc.tile_pool(name="st2", bufs=1) as st2:
                if has_mrb:
                    mrb_row = st2.tile([1, M], f32, tag="mrb")
                    dma.dma_start(mrb_row, mrb_d[:])
                moeT_sb = st2.tile([128, KH * 128], f16, tag="moeT")
                exp_sb = st2.tile([128, M], f32, tag="exp")
                expT_sb = st2.tile([128, 16 * 128], f16, tag="expT")
                s_sb = st2.tile([128, H], f32, tag="s")
                out_sb = st2.tile([128, H], f32, tag="o")
                srec = st2.tile([128, 1], f32, tag="srec")

                with tc.tile_pool(name="pt", bufs=1, space="PSUM") as pt:
                    with tc.tile_pool(name="plg", bufs=1, space="PSUM") as plg:
                        lg = [plg.tile([128, 512], f32, tag="lg", bufs=4,
                                       name=f"lg{n}") for n in range(4)]
                        for ch in range(NH):
                            tp = pt.tile([128, 512], f32, tag="tp", bufs=2,
                                         name=f"tpm{ch}")
                            for j in range(4):
                                hk = ch * 4 + j
                                nc.tensor.transpose(tp[:, j * 128:(j + 1) * 128],
                                                    moe_r[:, hk * 128:(hk + 1) * 128],
                                                    identity)
                            nc.scalar.copy(moeT_sb[:, ch * 512:(ch + 1) * 512], tp)
                            for j in range(4):
                                hk = ch * 4 + j
                                for n in range(4):
                                    nc.tensor.matmul(
                                        lg[n],
                                        moeT_sb[:, hk * 128:(hk + 1) * 128],
                                        mrw_tiles[hk][:, n * 512:(n + 1) * 512],
                                        start=(hk == 0),
                                        stop=(hk == KH - 1) and not has_mrb)
                        if has_mrb:
                            for n in range(4):
                                nc.tensor.matmul(lg[n], ones_row[0:1, 0:1],
                                                 mrb_row[0:1, n * 512:(n + 1) * 512],
                                                 start=False, stop=True)

                        nmx = []
                        for n in range(4):
                            t_ = st2.tile([128, 1], f32, tag=f"nmx{n}", name=f"nmx{n}")
                            nc.vector.reduce_max(t_, lg[n], axis=AX.X, negate=True)
                            nmx.append(t_)
                        t01 = st2.tile([128, 1], f32, tag="t01")
                        nc.vector.tensor_scalar_min(t01, nmx[0], nmx[1])
                        t23 = st2.tile([128, 1], f32, tag="t23")
                        nc.vector.tensor_scalar_min(t23, nmx[2], nmx[3])
                        ngm2 = st2.tile([128, 1], f32, tag="ngm2")
                        nc.vector.tensor_scalar_min(ngm2, t01, t23)
                        ses = []
                        for n in range(4):
                            se_ = st2.tile([128, 1], f32, tag=f"ses{n}", name=f"ses{n}")
                            nc.scalar.activation(exp_sb[:, n * 512:(n + 1) * 512],
                                                 lg[n], AF.Exp, bias=ngm2,
                                                 accum_out=se_)
                            ses.append(se_)
                        s01 = st2.tile([128, 1], f32, tag="s01")
                        nc.vector.tensor_tensor(s01, ses[0], ses[1], ALU.add)
                        s23 = st2.tile([128, 1], f32, tag="s23")
                        nc.vector.tensor_tensor(s23, ses[2], ses[3], ALU.add)
                        stot = st2.tile([128, 1], f32, tag="stot")
                        nc.vector.tensor_tensor(stot, s01, s23, ALU.add)
                        nc.vector.reciprocal(srec, stot)

                    for t in range(4):
                        tp = pt.tile([128, 512], f32, tag="tp", bufs=2, name=f"tpe{t}")
                        for j in range(4):
                            mk = t * 4 + j
                            nc.tensor.transpose(tp[:, j * 128:(j + 1) * 128],
                                                exp_sb[:, mk * 128:(mk + 1) * 128],
                                                identity)
                        nc.scalar.copy(expT_sb[:, t * 512:(t + 1) * 512], tp)

                with tc.tile_pool(name="prd", bufs=1, space="PSUM") as prd:
                    rd = [prd.tile([128, 512], f32, tag="rd", bufs=4,
                                   name=f"rd{n}") for n in range(4)]
                    for mk in range(16):
                        for n in range(4):
                            nc.tensor.matmul(rd[n],
                                             expT_sb[:, mk * 128:(mk + 1) * 128],
                                             mem_tiles[mk][:, n * 512:(n + 1) * 512],
                                             start=(mk == 0), stop=(mk == 15))
                    # s = moe + read_vec/sum  (deferred softmax normalization);
                    # columns >= h1 have moe == 0 by mask structure
                    for n in range(4):
                        sl = slice(n * 512, (n + 1) * 512)
                        if n < NH:
                            nc.vector.scalar_tensor_tensor(s_sb[:, sl], rd[n],
                                                           srec, moe_r[:, sl],
                                                           ALU.mult, ALU.add)
                        else:
                            nc.vector.tensor_scalar_mul(s_sb[:, sl], rd[n], srec)

                # blended learned activation via diag-matmul accumulation.
                # Mish is not in any HW act table; synthesize
                # mish(s) = s * tanh(relu(s) + ln(1 + exp(-|s|))).
                with tc.tile_pool(name="pac", bufs=1, space="PSUM") as pac, \
                     tc.tile_pool(name="brp", bufs=1) as brp:
                    acc = [pac.tile([128, 512], f32, tag="acc", bufs=4,
                                    name=f"acc{n}") for n in range(4)]
                    n_groups = 7
                    gi = [0]

                    def acc_branch(br_tile, ci):
                        diag = brp.tile([128, 128], f32r, tag="d", bufs=2,
                                        name=f"d{gi[0]}")
                        nc.vector.tensor_scalar_mul(diag, identity,
                                                    coeffs_bc[:, ci:ci + 1])
                        for n in range(4):
                            nc.tensor.matmul(acc[n], diag,
                                             br_tile[:, n * 512:(n + 1) * 512],
                                             start=(gi[0] == 0),
                                             stop=(gi[0] == n_groups - 1))
                        gi[0] += 1

                    # --- nl_exp table phase ---
                    relu_br = brp.tile([128, H], f32r, tag="relu")
                    nc.scalar.activation(relu_br, s_sb, AF.Relu)
                    acc_branch(relu_br, 5)
                    # exp(min(s,0)) branch; the -1 of expm1 is folded into the
                    # final subtraction of c_em below
                    mn = brp.tile([128, H], f32, tag="sc1", bufs=2, name="mn")
                    nc.vector.tensor_scalar_min(mn, s_sb, 0.0)
                    em_br = brp.tile([128, H], f32r, tag="b", bufs=2, name="em")
                    nc.scalar.activation(em_br, mn, AF.Exp)
                    acc_branch(em_br, 6)
                    # softplus(s) = relu(s) + ln(1 + exp(-|s|))
                    abs_s = brp.tile([128, H], f32, tag="sc2", bufs=2, name="ab")
                    nc.scalar.activation(abs_s, s_sb, AF.Abs)
                    enab = brp.tile([128, H], f32, tag="sc1", bufs=2, name="en")
                    nc.scalar.activation(enab, abs_s, AF.Exp, scale=-1.0)
                    ep1 = brp.tile([128, H], f32, tag="sc2", bufs=2, name="e1")
                    nc.vector.tensor_scalar_add(ep1, enab, 1.0)
                    ln1p = brp.tile([128, H], f32, tag="sc1", bufs=2, name="ln")
                    nc.scalar.activation(ln1p, ep1, AF.Ln)
                    sp = brp.tile([128, H], f32, tag="sp")
                    nc.vector.tensor_tensor(sp, ln1p, relu_br.bitcast(f32),
                                            ALU.add)
                    # --- sigmoid table phase (Sigmoid + Tanh) ---
                    sg_br = brp.tile([128, H], f32r, tag="b", bufs=2, name="sg")
                    nc.scalar.activation(sg_br, s_sb, AF.Sigmoid)
                    acc_branch(sg_br, 0)
                    th_br = brp.tile([128, H], f32r, tag="b", bufs=2, name="th")
                    nc.scalar.activation(th_br, s_sb, AF.Tanh)
                    acc_branch(th_br, 1)
                    mt = brp.tile([128, H], f32, tag="sc2", bufs=2, name="mt")
                    nc.scalar.activation(mt, sp, AF.Tanh)
                    mish_br = brp.tile([128, H], f32r, tag="b", bufs=2, name="mi")
                    nc.vector.tensor_tensor(mish_br, mt, s_sb, ALU.mult)
                    acc_branch(mish_br, 4)
                    # --- silu table phase ---
                    sl_br = brp.tile([128, H], f32r, tag="b", bufs=2, name="sl")
                    nc.scalar.activation(sl_br, s_sb, AF.Silu)
                    acc_branch(sl_br, 2)
                    # --- gelu table phase ---
                    gl_br = brp.tile([128, H], f32r, tag="b", bufs=2, name="gl")
                    nc.scalar.activation(gl_br, s_sb, AF.Gelu)
                    acc_branch(gl_br, 3)
                    assert gi[0] == n_groups
                    for n in range(4):
                        nc.vector.tensor_scalar_sub(out_sb[:, n * 512:(n + 1) * 512],
                                                    acc[n], coeffs_bc[:, 6:7])
                gdma.dma_start(out_d[:, :], out_sb)
    nc.finalize()
    return nc


def _get_nc(key=None):
    if key is None:
        key = _LAST_KEY
    if key not in _CACHED_NC:
        _CACHED_NC[key] = _build_program(*key)
    return _CACHED_NC[key]


def _r12(a):
    """Round fp32 to the fp32r grid (11 explicit mantissa bits, RNE)."""
    u = np.ascontiguousarray(a).view(np.uint32)
    u = (u + np.uint32(0x7FF) + ((u >> np.uint32(12)) & np.uint32(1))) \
        & np.uint32(0xFFFFF000)
    return u.view(np.float32)


def kernel(**inputs):
    import os
    from concourse.bass_utils import run_bass_kernel_spmd

    f = lambda a: np.ascontiguousarray(np.asarray(a, dtype=np.float32))
    x = f(inputs["x"])
    gate_w = f(inputs["gate_w"])
    expert_w = f(inputs["expert_w"])
    expert_b = f(inputs["expert_b"])
    conn_w1 = f(inputs["conn_w1"])
    conn_b1 = f(inputs["conn_b1"])
    conn_w2 = f(inputs["conn_w2"])
    conn_b2 = f(inputs["conn_b2"])
    neuron_avg = f(inputs["neuron_avg"])
    neuron_mask = f(inputs["neuron_mask"])
    mem_read_w = f(inputs["mem_read_w"])
    mem_read_b = f(inputs["mem_read_b"])
    memory = f(inputs["memory"])
    act_w = f(inputs["act_w"]).reshape(-1)

    g = int(os.environ.get("MOE_G", "2"))
    EL = E // g
    NB = g

    # host prep: softmax blend weights -> 7 branch coefficients
    p = np.exp(act_w - act_w.max())
    p = p / p.sum()
    coef = np.array([[p[0], p[2], p[4], p[5], p[7],
                      p[1] + p[3] + p[6] * SELU_SCALE,
                      p[1] + p[6] * SELU_SCALE * SELU_ALPHA, 0.0]], np.float32)

    # host conn MLP (batch-independent) -> cmask folded into expert weights
    h1v = np.einsum('eh,ehk->ek', neuron_avg, conn_w1) + conn_b1
    h1v = np.maximum(h1v, 0.0, dtype=np.float32)
    cl = np.einsum('ek,ekh->eh', h1v, conn_w2) + conn_b2
    conn = (1.0 / (1.0 + np.exp(-cl))).astype(np.float32)
    cmask = conn * neuron_mask                                   # [E, H]

    # stage-1 live width: columns past the last nonzero mask column are
    # structurally zero in moe_out, so the program skips them entirely
    nz = np.nonzero(neuron_mask.any(axis=0))[0]
    h1 = int(nz[-1]) + 1 if nz.size else 512
    h1 = min(H, max(512, -(-h1 // 512) * 512))

    wp = (expert_w[:, :, :h1] * cmask[:, None, :h1]).astype(np.float16)
    bp = (expert_b[:, :h1] * cmask[:, :h1]).astype(np.float16)
    has_eb = bool(np.any(bp))
    has_mrb = bool(np.any(mem_read_b))

    xT = np.ascontiguousarray(x.T).astype(np.float16)            # [D, B]
    xT_blk = xT.reshape(8, 128, B).transpose(1, 0, 2)            # [128, 8, B]
    mrw_bf = mem_read_w[:h1].reshape(h1 // 128, 128, M).astype(np.float16)
    mem_bf = memory.reshape(16, 128, H).astype(np.float16)
    mrb = np.ascontiguousarray(mem_read_b.reshape(1, M))

    in_maps = []
    for c in range(NCORES):
        if g > 1:
            bg, eg = c >> 1, c & 1
        else:
            bg, eg = c, 0
        gwr = np.roll(gate_w, -eg * EL, axis=1).astype(np.float16)
        ew_c = wp[eg * EL:(eg + 1) * EL]         # [EL, D, h1]
        m = {
            "xT": np.ascontiguousarray(
                xT_blk[:, :, bg * NB * 128:(bg + 1) * NB * 128]),
            "gw": np.ascontiguousarray(gwr.reshape(8, 128, E).transpose(1, 0, 2)),
            "ew": np.ascontiguousarray(
                ew_c.reshape(EL, 8, 128, h1).transpose(0, 2, 1, 3)),
            "mrw": mrw_bf,
            "mem": mem_bf,
            "coef": coef,
        }
        if has_eb:
            m["eb"] = np.ascontiguousarray(
                bp[eg * EL:(eg + 1) * EL].reshape(EL, 1, h1))
        if has_mrb:
            m["mrb"] = mrb
        in_maps.append(m)

    key = (h1, g, has_eb, has_mrb)
    global _LAST_IN_MAPS, _LAST_KEY
    _LAST_IN_MAPS = in_maps
    _LAST_KEY = key
    nc = _get_nc(key)
    results = run_bass_kernel_spmd(nc, in_maps, list(range(NCORES))).results
    out = np.concatenate(
        [np.asarray(results[c]["out"], dtype=np.float32) for c in range(NCORES)],
        axis=0)
    return out


# revision 5
# speedup vs baseline: 1.7109x; 1.0314x over previous
"""Hybrid expert/batch-parallel Trainium2 kernel for PlasticityModelMoE.

Sharding (g=2 expert-parallel x b=4 batch-parallel): core c = (batch group
bg=c>>1, expert group eg=c&1). Each core computes its 4 experts' gate-weighted
sum for its 256 batch rows, then ONE pairwise ReduceScatter(add) over
[[0,1],[2,3],[4,5],[6,7]] (512KB) leaves core c with batch rows
[128c, 128c+128). Stage 2 (episodic-memory attention + blended learned
activation) runs on those 128 rows with replicated mem_read_w / memory in
bf16. Host concatenates the 8 [128, 2048] outputs.

Host-side folding: the DynamicConnectivity MLP depends only on neuron_avg
(batch-independent), so cmask = sigmoid(conn)*neuron_mask is computed on the
host and folded into expert_w columns (relu(z*m) == m*relu(z) for m>=0);
device stage 1 is then just gate softmax + z matmuls + relu/gate-scale
accumulation. Columns past the last nonzero mask column are structurally zero
in moe_out, so only h1 columns are computed (and only h1 rows of mem_read_w
are loaded).

DMA rings: sync + vector stream the folded expert weights (half-expert
tiles); scalar(ACT) prefetches mem_read_w then memory (bf16); gpsimd carries
xT/gw/coef, the RS traffic, and the output.
"""

import numpy as np

B, D, H, E, M = 1024, 1024, 2048, 8, 2048
NCORES = 8
SELU_SCALE = 1.0507009873554805
SELU_ALPHA = 1.6732632423543772

_CACHED_NC = {}
_LAST_KEY = None
_LAST_IN_MAPS = None


def _build_program(h1, g, has_eb, has_mrb):
    import concourse.bass as bass
    from concourse import bacc, mybir, tile
    from concourse.masks import make_identity

    f32 = mybir.dt.float32
    f32r = mybir.dt.float32r
    f16 = mybir.dt.float16
    EL = E // g          # experts per core
    NB = g               # 128-row batch blocks per core
    NH = h1 // 512       # stage-1 column chunks
    KH = h1 // 128       # K blocks for the attention logits
    AF = mybir.ActivationFunctionType
    ALU = mybir.AluOpType
    AX = mybir.AxisListType

    nc = bacc.Bacc(None, target_bir_lowering=False, debug=False)

    xT_d = nc.dram_tensor("xT", [128, 8, NB * 128], f16, kind="ExternalInput")
    gw_d = nc.dram_tensor("gw", [128, 8, E], f16, kind="ExternalInput")
    ew_d = nc.dram_tensor("ew", [EL, 128, 8, h1], f16, kind="ExternalInput")
    if has_eb:
        eb_d = nc.dram_tensor("eb", [EL, 1, h1], f16, kind="ExternalInput")
    mrw_d = nc.dram_tensor("mrw", [KH, 128, M], f16, kind="ExternalInput")
    if has_mrb:
        mrb_d = nc.dram_tensor("mrb", [1, M], f32, kind="ExternalInput")
    mem_d = nc.dram_tensor("mem", [16, 128, H], f16, kind="ExternalInput")
    cf_d = nc.dram_tensor("coef", [1, 8], f32, kind="ExternalInput")
    out_d = nc.dram_tensor("out", [128, H], f32, kind="ExternalOutput")

    dma = nc.default_dma_engine   # SP hwdge ring: ew h0 + mem[0:8]
    adma = nc.scalar              # ACT ring: ew h1 + mrw + mem[8:16]
    gdma = nc.gpsimd              # gpsimd ring: xT/gw/coef + RS + out

    with tile.TileContext(nc) as tc:
        with tc.tile_pool(name="consts", bufs=1) as consts, \
             tc.tile_pool(name="dramp", bufs=1, space="DRAM") as dramp, \
             tc.tile_pool(name="mrwp", bufs=KH) as mrwp, \
             tc.tile_pool(name="memp", bufs=16) as memp:

            identity = consts.tile([128, 128], f32, tag="idn")
            make_identity(nc, identity)
            ones_row = consts.tile([1, 128], f32, tag="ones")
            nc.vector.memset(ones_row, 1.0)
            coef_row = consts.tile([1, 8], f32, tag="coef")
            gdma.dma_start(coef_row, cf_d[:])
            coeffs_bc = consts.tile([128, 8], f32, tag="cfb")
            moe_r = consts.tile([128, h1], f32, tag="moer")


            if g > 1:
                ys = [dramp.tile([NB * 128, 512], f32, tag=f"y{n}", name=f"y{n}")
                      for n in range(NH)]
                rss = [dramp.tile([128, 512], f32, tag=f"rs{n}", name=f"rs{n}")
                       for n in range(NH)]
                groups = [[2 * k, 2 * k + 1] for k in range(4)]
                # warm up the CC stream (absorbs barrier + first-trigger cost)
                dum_i = dramp.tile([128, 4], f32, tag="dmi", name="dmi")
                dum_o = dramp.tile([64, 4], f32, tag="dmo", name="dmo")
                nc.gpsimd.collective_compute(
                    "ReduceScatter", bass.mybir.AluOpType.add,
                    replica_groups=groups,
                    ins=[dum_i.opt()], outs=[dum_o.opt()])

            # ---------------- stage 1: hybrid-parallel MoE ----------------
            with tc.tile_pool(name="w1", bufs=1) as w1, \
                 tc.tile_pool(name="ewp", bufs=8) as ewp, \
                 tc.tile_pool(name="pb", bufs=1, space="PSUM") as pb:
                xT_sb = w1.tile([128, 8, NB * 128], f16, tag="xT")
                gdma.dma_start(xT_sb, xT_d[:])
                gw_sb = w1.tile([128, 8, E], f16, tag="gw")
                gdma.dma_start(gw_sb, gw_d[:])
                if has_eb:
                    eb_rows = w1.tile([EL, 1, h1], f16, tag="eb")
                    dma.dma_start(eb_rows, eb_d[:])
                    ones_f16 = w1.tile([1, 128], f16, tag="o16")
                    nc.vector.memset(ones_f16, 1.0)

                # expert weight stream: half-expert tiles on two rings
                ew_tiles = []
                for e in range(EL):
                    t0 = ewp.tile([128, 4, h1], f16, tag="ew", name=f"ew{e}h0")
                    dma.dma_start(t0, ew_d[e, :, 0:4, :])
                    t1 = ewp.tile([128, 4, h1], f16, tag="ew", name=f"ew{e}h1")
                    adma.dma_start(t1, ew_d[e, :, 4:8, :])
                    ew_tiles.append((t0, t1))

                # stage-2 weights queued behind the expert stream
                mrw_tiles = []
                for hk in range(KH):
                    t_ = mrwp.tile([128, M], f16, tag="w", name=f"mrw{hk}")
                    adma.dma_start(t_, mrw_d[hk])
                    mrw_tiles.append(t_)
                mem_tiles = []
                for mk in range(16):
                    t_ = memp.tile([128, H], f16, tag="m", name=f"mem{mk}")
                    (dma if mk < 8 else adma).dma_start(t_, mem_d[mk])
                    mem_tiles.append(t_)

                # broadcast activation-blend coefficients to 128 partitions
                cf_ps = pb.tile([128, 8], f32, tag="cf")
                nc.tensor.matmul(cf_ps, ones_row, coef_row, start=True, stop=True)
                nc.scalar.copy(coeffs_bc, cf_ps)

                # gate softmax per batch block -> per-own-expert columns
                gcols = []
                for i in range(NB):
                    bs = slice(i * 128, (i + 1) * 128)
                    gate_ps = pb.tile([128, E], f32, tag="g", bufs=2, name=f"g{i}")
                    for k in range(8):
                        nc.tensor.matmul(gate_ps, xT_sb[:, k, bs], gw_sb[:, k, :],
                                         start=(k == 0), stop=(k == 7))
                    ngm = w1.tile([128, 1], f32, tag="ngm", bufs=2, name=f"ngm{i}")
                    nc.vector.reduce_max(ngm, gate_ps, axis=AX.X, negate=True)
                    eg_t = w1.tile([128, E], f32, tag="eg", bufs=2, name=f"eg{i}")
                    sume = w1.tile([128, 1], f32, tag="se", bufs=2, name=f"se{i}")
                    nc.scalar.activation(eg_t, gate_ps, AF.Exp, bias=ngm,
                                         accum_out=sume)
                    rec = w1.tile([128, 1], f32, tag="rec", bufs=2, name=f"rec{i}")
                    nc.vector.reciprocal(rec, sume)
                    cols = []
                    for j in range(EL):
                        gcol = w1.tile([128, 1], f32, tag=f"gc{i}_{j}",
                                       name=f"gc{i}_{j}")
                        nc.vector.tensor_scalar_mul(gcol, eg_t[:, j:j + 1], rec)
                        cols.append(gcol)
                    gcols.append(cols)

                moe_parts = [moe_r] if g == 1 else [
                    w1.tile([128, h1], f32, tag=f"mp{i}", name=f"mp{i}")
                    for i in range(NB)]

                for n in range(NH):
                    sl = slice(n * 512, (n + 1) * 512)
                    for e in range(EL):
                        t0, t1 = ew_tiles[e]
                        for i in range(NB):
                            bs = slice(i * 128, (i + 1) * 128)
                            z_ps = pb.tile([128, 512], f32, tag="z", bufs=4,
                                           name=f"z{e}_{n}_{i}")
                            for k in range(4):
                                nc.tensor.matmul(z_ps, xT_sb[:, k, bs],
                                                 t0[:, k, sl],
                                                 start=(k == 0), stop=False)
                            for k in range(4):
                                last = (k == 3) and not has_eb
                                nc.tensor.matmul(z_ps, xT_sb[:, 4 + k, bs],
                                                 t1[:, k, sl],
                                                 start=False, stop=last)
                            if has_eb:
                                nc.tensor.matmul(z_ps, ones_f16[0:1, 0:1],
                                                 eb_rows[e, 0:1, sl],
                                                 start=False, stop=True)
                            if e == 0:
                                nc.scalar.activation(moe_parts[i][:, sl], z_ps,
                                                     AF.Relu, scale=gcols[i][e])
                            else:
                                t_ = w1.tile([128, 512], f32, tag="acc", bufs=3,
                                             name=f"a{e}_{n}_{i}")
                                nc.scalar.activation(t_, z_ps, AF.Relu,
                                                     scale=gcols[i][e])
                                nc.vector.tensor_tensor(moe_parts[i][:, sl],
                                                        moe_parts[i][:, sl],
                                                        t_, ALU.add)

                    # pairwise ReduceScatter as soon as the chunk is done
                    if g > 1:
                        for i in range(NB):
                            gdma.dma_start(ys[n][i * 128:(i + 1) * 128, :],
                                           moe_parts[i][:, sl])
                        nc.gpsimd.collective_compute(
                            "ReduceScatter",
                            bass.mybir.AluOpType.add,
                            replica_groups=groups,
                            ins=[ys[n].opt()],
                            outs=[rss[n].opt()],
                        )
                        gdma.dma_start(moe_r[:, sl], rss[n])

            # ---------------- stage 2: memory read + learned activation ------
            with tc.tile_pool(name="st2", bufs=1) as st2:
                if has_mrb:
                    mrb_row = st2.tile([1, M], f32, tag="mrb")
                    dma.dma_start(mrb_row, mrb_d[:])
                moeT_sb = st2.tile([128, KH * 128], f16, tag="moeT")
                exp_sb = st2.tile([128, M], f32, tag="exp")
                expT_sb = st2.tile([128, 16 * 128], f16, tag="expT")
                s_sb = st2.tile([128, H], f32, tag="s")
                out_sb = st2.tile([128, H], f32, tag="o")
                srec = st2.tile([128, 1], f32, tag="srec")

                with tc.tile_pool(name="pt", bufs=1, space="PSUM") as pt:
                    with tc.tile_pool(name="plg", bufs=1, space="PSUM") as plg:
                        lg = [plg.tile([128, 512], f32, tag="lg", bufs=4,
                                       name=f"lg{n}") for n in range(4)]
                        for ch in range(NH):
                            tp = pt.tile([128, 512], f32, tag="tp", bufs=2,
                                         name=f"tpm{ch}")
                            for j in range(4):
                                hk = ch * 4 + j
                                nc.tensor.transpose(tp[:, j * 128:(j + 1) * 128],
                                                    moe_r[:, hk * 128:(hk + 1) * 128],
                                                    identity)
                            nc.scalar.copy(moeT_sb[:, ch * 512:(ch + 1) * 512], tp)
                            for j in range(4):
                                hk = ch * 4 + j
                                for n in range(4):
                                    nc.tensor.matmul(
                                        lg[n],
                                        moeT_sb[:, hk * 128:(hk + 1) * 128],
                                        mrw_tiles[hk][:, n * 512:(n + 1) * 512],
                                        start=(hk == 0),
                                        stop=(hk == KH - 1) and not has_mrb)
                        if has_mrb:
                            for n in range(4):
                                nc.tensor.matmul(lg[n], ones_row[0:1, 0:1],
                                                 mrb_row[0:1, n * 512:(n + 1) * 512],
                                                 start=False, stop=True)

                        nmx = []
                        for n in range(4):
                            t_ = st2.tile([128, 1], f32, tag=f"nmx{n}", name=f"nmx{n}")
                            nc.vector.reduce_max(t_, lg[n], axis=AX.X, negate=True)
                            nmx.append(t_)
                        t01 = st2.tile([128, 1], f32, tag="t01")
                        nc.vector.tensor_scalar_min(t01, nmx[0], nmx[1])
                        t23 = st2.tile([128, 1], f32, tag="t23")
                        nc.vector.tensor_scalar_min(t23, nmx[2], nmx[3])
                        ngm2 = st2.tile([128, 1], f32, tag="ngm2")
                        nc.vector.tensor_scalar_min(ngm2, t01, t23)
                        ses = []
                        for n in range(4):
                            se_ = st2.tile([128, 1], f32, tag=f"ses{n}", name=f"ses{n}")
                            nc.scalar.activation(exp_sb[:, n * 512:(n + 1) * 512],
                                                 lg[n], AF.Exp, bias=ngm2,
                                                 accum_out=se_)
                            ses.append(se_)
                        s01 = st2.tile([128, 1], f32, tag="s01")
                        nc.vector.tensor_tensor(s01, ses[0], ses[1], ALU.add)
                        s23 = st2.tile([128, 1], f32, tag="s23")
                        nc.vector.tensor_tensor(s23, ses[2], ses[3], ALU.add)
                        stot = st2.tile([128, 1], f32, tag="stot")
                        nc.vector.tensor_tensor(stot, s01, s23, ALU.add)
                        nc.vector.reciprocal(srec, stot)

                    for t in range(4):
                        tp = pt.tile([128, 512], f32, tag="tp", bufs=2, name=f"tpe{t}")
                        for j in range(4):
                            mk = t * 4 + j
                            nc.tensor.transpose(tp[:, j * 128:(j + 1) * 128],
                                                exp_sb[:, mk * 128:(mk + 1) * 128],
                                                identity)
                        nc.scalar.copy(expT_sb[:, t * 512:(t + 1) * 512], tp)

                with tc.tile_pool(name="prd", bufs=1, space="PSUM") as prd:
                    rd = [prd.tile([128, 512], f32, tag="rd", bufs=4,
                                   name=f"rd{n}") for n in range(4)]
                    for mk in range(16):
                        for n in range(4):
                            nc.tensor.matmul(rd[n],
                                             expT_sb[:, mk * 128:(mk + 1) * 128],
                                             mem_tiles[mk][:, n * 512:(n + 1) * 512],
                                             start=(mk == 0), stop=(mk == 15))
                    # s = moe + read_vec/sum  (deferred softmax normalization);
                    # columns >= h1 have moe == 0 by mask structure
                    for n in range(4):
                        sl = slice(n * 512, (n + 1) * 512)
                        if n < NH:
                            nc.vector.scalar_tensor_tensor(s_sb[:, sl], rd[n],
                                                           srec, moe_r[:, sl],
                                                           ALU.mult, ALU.add)
                        else:
                            nc.vector.tensor_scalar_mul(s_sb[:, sl], rd[n], srec)

                # blended learned activation via diag-matmul accumulation.
                # Mish is not in any HW act table; synthesize
                # mish(s) = s * tanh(relu(s) + ln(1 + exp(-|s|))).
                with tc.tile_pool(name="pac", bufs=1, space="PSUM") as pac, \
                     tc.tile_pool(name="brp", bufs=1) as brp:
                    acc = [pac.tile([128, 512], f32, tag="acc", bufs=4,
                                    name=f"acc{n}") for n in range(4)]
                    n_groups = 7
                    gi = [0]

                    def acc_branch(br_tile, ci):
                        diag = brp.tile([128, 128], f32r, tag="d", bufs=2,
                                        name=f"d{gi[0]}")
                        nc.vector.tensor_scalar_mul(diag, identity,
                                                    coeffs_bc[:, ci:ci + 1])
                        for n in range(4):
                            nc.tensor.matmul(acc[n], diag,
                                             br_tile[:, n * 512:(n + 1) * 512],
                                             start=(gi[0] == 0),
                                             stop=(gi[0] == n_groups - 1))
                        gi[0] += 1

                    # --- nl_exp table phase ---
                    relu_br = brp.tile([128, H], f32r, tag="relu")
                    nc.scalar.activation(relu_br, s_sb, AF.Relu)
                    acc_branch(relu_br, 5)
                    # exp(min(s,0)) branch; the -1 of expm1 is folded into the
                    # final subtraction of c_em below
                    mn = brp.tile([128, H], f32, tag="sc1", bufs=2, name="mn")
                    nc.vector.tensor_scalar_min(mn, s_sb, 0.0)
                    em_br = brp.tile([128, H], f32r, tag="b", bufs=2, name="em")
                    nc.scalar.activation(em_br, mn, AF.Exp)
                    acc_branch(em_br, 6)
                    # softplus(s) = relu(s) + ln(1 + exp(-|s|))
                    abs_s = brp.tile([128, H], f32, tag="sc2", bufs=2, name="ab")
                    nc.scalar.activation(abs_s, s_sb, AF.Abs)
                    enab = brp.tile([128, H], f32, tag="sc1", bufs=2, name="en")
                    nc.scalar.activation(enab, abs_s, AF.Exp, scale=-1.0)
                    ep1 = brp.tile([128, H], f32, tag="sc2", bufs=2, name="e1")
                    nc.vector.tensor_scalar_add(ep1, enab, 1.0)
                    ln1p = brp.tile([128, H], f32, tag="sc1", bufs=2, name="ln")
                    nc.scalar.activation(ln1p, ep1, AF.Ln)
                    sp = brp.tile([128, H], f32, tag="sp")
                    nc.vector.tensor_tensor(sp, ln1p, relu_br.bitcast(f32),
                                            ALU.add)
                    # --- sigmoid table phase (Sigmoid + Tanh) ---
                    sg_br = brp.tile([128, H], f32r, tag="b", bufs=2, name="sg")
                    nc.scalar.activation(sg_br, s_sb, AF.Sigmoid)
                    acc_branch(sg_br, 0)
                    th_br = brp.tile([128, H], f32r, tag="b", bufs=2, name="th")
                    nc.scalar.activation(th_br, s_sb, AF.Tanh)
                    acc_branch(th_br, 1)
                    mt = brp.tile([128, H], f32, tag="sc2", bufs=2, name="mt")
                    nc.scalar.activation(mt, sp, AF.Tanh)
                    mish_br = brp.tile([128, H], f32r, tag="b", bufs=2, name="mi")
                    nc.vector.tensor_tensor(mish_br, mt, s_sb, ALU.mult)
                    acc_branch(mish_br, 4)
                    # silu = s * sigmoid(s), on the vector engine
                    sl_br = brp.tile([128, H], f32r, tag="b", bufs=2, name="sl")
                    nc.vector.tensor_tensor(sl_br, sg_br.bitcast(f32), s_sb,
                                            ALU.mult)
                    acc_branch(sl_br, 2)
                    # --- gelu table phase ---
                    gl_br = brp.tile([128, H], f32r, tag="b", bufs=2, name="gl")
                    nc.scalar.activation(gl_br, s_sb, AF.Gelu)
                    acc_branch(gl_br, 3)
                    assert gi[0] == n_groups
                    for n in range(4):
                        nc.vector.tensor_scalar_sub(out_sb[:, n * 512:(n + 1) * 512],
                                                    acc[n], coeffs_bc[:, 6:7])
                gdma.dma_start(out_d[:, :], out_sb)
    nc.finalize()
    return nc


def _get_nc(key=None):
    if key is None:
        key = _LAST_KEY
    if key not in _CACHED_NC:
        _CACHED_NC[key] = _build_program(*key)
    return _CACHED_NC[key]


def _r12(a):
    """Round fp32 to the fp32r grid (11 explicit mantissa bits, RNE)."""
    u = np.ascontiguousarray(a).view(np.uint32)
    u = (u + np.uint32(0x7FF) + ((u >> np.uint32(12)) & np.uint32(1))) \
        & np.uint32(0xFFFFF000)
    return u.view(np.float32)


def kernel(**inputs):
    import os
    from concourse.bass_utils import run_bass_kernel_spmd

    f = lambda a: np.ascontiguousarray(np.asarray(a, dtype=np.float32))
    x = f(inputs["x"])
    gate_w = f(inputs["gate_w"])
    expert_w = f(inputs["expert_w"])
    expert_b = f(inputs["expert_b"])
    conn_w1 = f(inputs["conn_w1"])
    conn_b1 = f(inputs["conn_b1"])
    conn_w2 = f(inputs["conn_w2"])
    conn_b2 = f(inputs["conn_b2"])
    neuron_avg = f(inputs["neuron_avg"])
    neuron_mask = f(inputs["neuron_mask"])
    mem_read_w = f(inputs["mem_read_w"])
    mem_read_b = f(inputs["mem_read_b"])
    memory = f(inputs["memory"])
    act_w = f(inputs["act_w"]).reshape(-1)

    g = int(os.environ.get("MOE_G", "2"))
    EL = E // g
    NB = g

    # host prep: softmax blend weights -> 7 branch coefficients
    p = np.exp(act_w - act_w.max())
    p = p / p.sum()
    coef = np.array([[p[0], p[2], p[4], p[5], p[7],
                      p[1] + p[3] + p[6] * SELU_SCALE,
                      p[1] + p[6] * SELU_SCALE * SELU_ALPHA, 0.0]], np.float32)

    # host conn MLP (batch-independent) -> cmask folded into expert weights
    h1v = np.einsum('eh,ehk->ek', neuron_avg, conn_w1) + conn_b1
    h1v = np.maximum(h1v, 0.0, dtype=np.float32)
    cl = np.einsum('ek,ekh->eh', h1v, conn_w2) + conn_b2
    conn = (1.0 / (1.0 + np.exp(-cl))).astype(np.float32)
    cmask = conn * neuron_mask                                   # [E, H]

    # stage-1 live width: columns past the last nonzero mask column are
    # structurally zero in moe_out, so the program skips them entirely
    nz = np.nonzero(neuron_mask.any(axis=0))[0]
    h1 = int(nz[-1]) + 1 if nz.size else 512
    h1 = min(H, max(512, -(-h1 // 512) * 512))

    wp = (expert_w[:, :, :h1] * cmask[:, None, :h1]).astype(np.float16)
    bp = (expert_b[:, :h1] * cmask[:, :h1]).astype(np.float16)
    has_eb = bool(np.any(bp))
    has_mrb = bool(np.any(mem_read_b))

    xT = np.ascontiguousarray(x.T).astype(np.float16)            # [D, B]
    xT_blk = xT.reshape(8, 128, B).transpose(1, 0, 2)            # [128, 8, B]
    mrw_bf = mem_read_w[:h1].reshape(h1 // 128, 128, M).astype(np.float16)
    mem_bf = memory.reshape(16, 128, H).astype(np.float16)
    mrb = np.ascontiguousarray(mem_read_b.reshape(1, M))

    in_maps = []
    for c in range(NCORES):
        if g > 1:
            bg, eg = c >> 1, c & 1
        else:
            bg, eg = c, 0
        gwr = np.roll(gate_w, -eg * EL, axis=1).astype(np.float16)
        ew_c = wp[eg * EL:(eg + 1) * EL]         # [EL, D, h1]
        m = {
            "xT": np.ascontiguousarray(
                xT_blk[:, :, bg * NB * 128:(bg + 1) * NB * 128]),
            "gw": np.ascontiguousarray(gwr.reshape(8, 128, E).transpose(1, 0, 2)),
            "ew": np.ascontiguousarray(
                ew_c.reshape(EL, 8, 128, h1).transpose(0, 2, 1, 3)),
            "mrw": mrw_bf,
            "mem": mem_bf,
            "coef": coef,
        }
        if has_eb:
            m["eb"] = np.ascontiguousarray(
                bp[eg * EL:(eg + 1) * EL].reshape(EL, 1, h1))
        if has_mrb:
            m["mrb"] = mrb
        in_maps.append(m)

    key = (h1, g, has_eb, has_mrb)
    global _LAST_IN_MAPS, _LAST_KEY
    _LAST_IN_MAPS = in_maps
    _LAST_KEY = key
    nc = _get_nc(key)
    results = run_bass_kernel_spmd(nc, in_maps, list(range(NCORES))).results
    out = np.concatenate(
        [np.asarray(results[c]["out"], dtype=np.float32) for c in range(NCORES)],
        axis=0)
    return out


# revision 9
# speedup vs baseline: 1.7689x; 1.0338x over previous
"""Hybrid expert/batch-parallel Trainium2 kernel for PlasticityModelMoE.

Sharding (g=2 expert-parallel x b=4 batch-parallel): core c = (batch group
bg=c>>1, expert group eg=c&1). Each core computes its 4 experts' gate-weighted
sum for its 256 batch rows, then ONE pairwise ReduceScatter(add) over
[[0,1],[2,3],[4,5],[6,7]] (512KB) leaves core c with batch rows
[128c, 128c+128). Stage 2 (episodic-memory attention + blended learned
activation) runs on those 128 rows with replicated mem_read_w / memory in
bf16. Host concatenates the 8 [128, 2048] outputs.

Host-side folding: the DynamicConnectivity MLP depends only on neuron_avg
(batch-independent), so cmask = sigmoid(conn)*neuron_mask is computed on the
host and folded into expert_w columns (relu(z*m) == m*relu(z) for m>=0);
device stage 1 is then just gate softmax + z matmuls + relu/gate-scale
accumulation. Columns past the last nonzero mask column are structurally zero
in moe_out, so only h1 columns are computed (and only h1 rows of mem_read_w
are loaded).

DMA rings: sync + vector stream the folded expert weights (half-expert
tiles); scalar(ACT) prefetches mem_read_w then memory (bf16); gpsimd carries
xT/gw/coef, the RS traffic, and the output.
"""

import numpy as np

B, D, H, E, M = 1024, 1024, 2048, 8, 2048
NCORES = 8
SELU_SCALE = 1.0507009873554805
SELU_ALPHA = 1.6732632423543772

_CACHED_NC = {}
_LAST_KEY = None
_LAST_IN_MAPS = None


def _build_program(h1, g, has_eb, has_mrb):
    import concourse.bass as bass
    from concourse import bacc, mybir, tile
    from concourse.masks import make_identity

    f32 = mybir.dt.float32
    f32r = mybir.dt.float32r
    f16 = mybir.dt.float16
    EL = E // g          # experts per core
    NB = g               # 128-row batch blocks per core
    NH = h1 // 512       # stage-1 column chunks
    KH = h1 // 128       # K blocks for the attention logits
    AF = mybir.ActivationFunctionType
    ALU = mybir.AluOpType
    AX = mybir.AxisListType

    nc = bacc.Bacc(None, target_bir_lowering=False, debug=False)

    xT_d = nc.dram_tensor("xT", [128, 8, NB * 128], f16, kind="ExternalInput")
    gw_d = nc.dram_tensor("gw", [128, 8, E], f16, kind="ExternalInput")
    ew_d = nc.dram_tensor("ew", [EL, 128, 8, h1], f16, kind="ExternalInput")
    if has_eb:
        eb_d = nc.dram_tensor("eb", [EL, 1, h1], f16, kind="ExternalInput")
    mrw_d = nc.dram_tensor("mrw", [KH, 128, M], f16, kind="ExternalInput")
    if has_mrb:
        mrb_d = nc.dram_tensor("mrb", [1, M], f32, kind="ExternalInput")
    mem_d = nc.dram_tensor("mem", [16, 128, H], f16, kind="ExternalInput")
    cf_d = nc.dram_tensor("coef", [1, 8], f32, kind="ExternalInput")
    out_d = nc.dram_tensor("out", [128, H], f32, kind="ExternalOutput")

    # sync carries the early-critical + most bulk traffic (it has no compute
    # duties so its in-order dma_start queue can block freely); scalar gets a
    # short queue so epilogue ACTs are never blocked behind DMA; gpsimd (slow
    # SW DGE) gets RS traffic, the output, and residual mem tiles.
    dma = nc.default_dma_engine   # SP hwdge ring
    adma = nc.scalar              # ACT hwdge ring
    gdma = nc.gpsimd              # gpsimd SW ring

    with tile.TileContext(nc) as tc:
        with tc.tile_pool(name="consts", bufs=1) as consts, \
             tc.tile_pool(name="dramp", bufs=1, space="DRAM") as dramp, \
             tc.tile_pool(name="mrwp", bufs=KH) as mrwp, \
             tc.tile_pool(name="memp", bufs=16) as memp:

            identity = consts.tile([128, 128], f32, tag="idn")
            make_identity(nc, identity)
            ones_row = consts.tile([1, 128], f32, tag="ones")
            nc.vector.memset(ones_row, 1.0)
            coef_row = consts.tile([1, 8], f32, tag="coef")
            dma.dma_start(coef_row, cf_d[:])
            coeffs_bc = consts.tile([128, 8], f32, tag="cfb")
            moe_r = consts.tile([128, h1], f32, tag="moer")


            if g > 1:
                ys = [dramp.tile([NB * 128, 512], f32, tag=f"y{n}", name=f"y{n}")
                      for n in range(NH)]
                rss = [dramp.tile([128, 512], f32, tag=f"rs{n}", name=f"rs{n}")
                       for n in range(NH)]
                groups = [[2 * k, 2 * k + 1] for k in range(4)]

            # ---------------- stage 1: hybrid-parallel MoE ----------------
            with tc.tile_pool(name="w1", bufs=1) as w1, \
                 tc.tile_pool(name="ewp", bufs=(8 if g > 1 else 4)) as ewp, \
                 tc.tile_pool(name="pb", bufs=1, space="PSUM") as pb:
                xT_sb = w1.tile([128, 8, NB * 128], f16, tag="xT")
                dma.dma_start(xT_sb, xT_d[:])
                gw_sb = w1.tile([128, 8, E], f16, tag="gw")
                adma.dma_start(gw_sb, gw_d[:])
                if has_eb:
                    eb_rows = w1.tile([EL, 1, h1], f16, tag="eb")
                    dma.dma_start(eb_rows, eb_d[:])
                    ones_f16 = w1.tile([1, 128], f16, tag="o16")
                    nc.vector.memset(ones_f16, 1.0)

                # gate softmax FIRST in scalar program order so its Exp
                # is never stuck behind bulk dma_starts on the ACT queue
                gcols = []
                for i in range(NB):
                    bs = slice(i * 128, (i + 1) * 128)
                    gate_ps = pb.tile([128, E], f32, tag="g", bufs=2, name=f"g{i}")
                    for k in range(8):
                        nc.tensor.matmul(gate_ps, xT_sb[:, k, bs], gw_sb[:, k, :],
                                         start=(k == 0), stop=(k == 7))
                    ngm = w1.tile([128, 1], f32, tag="ngm", bufs=2, name=f"ngm{i}")
                    nc.vector.reduce_max(ngm, gate_ps, axis=AX.X, negate=True)
                    eg_t = w1.tile([128, E], f32, tag="eg", bufs=2, name=f"eg{i}")
                    sume = w1.tile([128, 1], f32, tag="se", bufs=2, name=f"se{i}")
                    nc.scalar.activation(eg_t, gate_ps, AF.Exp, bias=ngm,
                                         accum_out=sume)
                    rec = w1.tile([128, 1], f32, tag="rec", bufs=2, name=f"rec{i}")
                    nc.vector.reciprocal(rec, sume)
                    cols = []
                    for j in range(EL):
                        gcol = w1.tile([128, 1], f32, tag=f"gc{i}_{j}",
                                       name=f"gc{i}_{j}")
                        nc.vector.tensor_scalar_mul(gcol, eg_t[:, j:j + 1], rec)
                        cols.append(gcol)
                    gcols.append(cols)

                # expert weight stream: half-expert tiles on two rings
                ew_tiles = []
                for e in range(EL):
                    t0 = ewp.tile([128, 4, h1], f16, tag="ew", name=f"ew{e}h0")
                    dma.dma_start(t0, ew_d[e, :, 0:4, :])
                    t1 = ewp.tile([128, 4, h1], f16, tag="ew", name=f"ew{e}h1")
                    adma.dma_start(t1, ew_d[e, :, 4:8, :])
                    ew_tiles.append((t0, t1))

                # stage-2 weights queued behind the expert stream, split so
                # each ring drains in time for its first consumer
                mrw_tiles = []
                for hk in range(KH):
                    t_ = mrwp.tile([128, M], f16, tag="w", name=f"mrw{hk}")
                    (dma if hk < 4 else adma).dma_start(t_, mrw_d[hk])
                    mrw_tiles.append(t_)
                mem_tiles = []
                for mk in range(16):
                    t_ = memp.tile([128, H], f16, tag="m", name=f"mem{mk}")
                    if g == 1:
                        eng = dma if mk < 5 else (adma if mk < 10 else gdma)
                    else:
                        eng = dma if mk < 6 else (adma if mk < 12 else gdma)
                    eng.dma_start(t_, mem_d[mk])
                    mem_tiles.append(t_)

                # broadcast activation-blend coefficients to 128 partitions
                cf_ps = pb.tile([128, 8], f32, tag="cf")
                nc.tensor.matmul(cf_ps, ones_row, coef_row, start=True, stop=True)
                nc.vector.tensor_copy(coeffs_bc, cf_ps)

                moe_parts = [moe_r] if g == 1 else [
                    w1.tile([128, h1], f32, tag=f"mp{i}", name=f"mp{i}")
                    for i in range(NB)]

                # g>1: chunk-major (all ew resident) so the RS for chunk n
                # fires mid-compute; g==1: expert-major streaming (no RS, and
                # 16 resident half-tiles would not fit SBUF)
                loop_order = ([(n, e) for n in range(NH) for e in range(EL)]
                              if g > 1 else
                              [(n, e) for e in range(EL) for n in range(NH)])
                for n, e in loop_order:
                        sl = slice(n * 512, (n + 1) * 512)
                        t0, t1 = ew_tiles[e]
                        for i in range(NB):
                            bs = slice(i * 128, (i + 1) * 128)
                            z_ps = pb.tile([128, 512], f32, tag="z", bufs=4,
                                           name=f"z{e}_{n}_{i}")
                            for k in range(4):
                                nc.tensor.matmul(z_ps, xT_sb[:, k, bs],
                                                 t0[:, k, sl],
                                                 start=(k == 0), stop=False)
                            for k in range(4):
                                last = (k == 3) and not has_eb
                                nc.tensor.matmul(z_ps, xT_sb[:, 4 + k, bs],
                                                 t1[:, k, sl],
                                                 start=False, stop=last)
                            if has_eb:
                                nc.tensor.matmul(z_ps, ones_f16[0:1, 0:1],
                                                 eb_rows[e, 0:1, sl],
                                                 start=False, stop=True)
                            # relu + gate-scale + accumulate, all on DVE so
                            # the ACT queue never gates PSUM recycling
                            t_ = w1.tile([128, 512], f32, tag="acc", bufs=3,
                                         name=f"a{e}_{n}_{i}")
                            nc.vector.tensor_scalar_max(t_, z_ps, 0.0)
                            if e == 0:
                                nc.vector.tensor_scalar_mul(
                                    moe_parts[i][:, sl], t_, gcols[i][e])
                            else:
                                nc.vector.scalar_tensor_tensor(
                                    moe_parts[i][:, sl], t_, gcols[i][e],
                                    moe_parts[i][:, sl], ALU.mult, ALU.add)

                        # pairwise ReduceScatter as soon as chunk n done
                        if g > 1 and e == EL - 1:
                            for i in range(NB):
                                gdma.dma_start(
                                    ys[n][i * 128:(i + 1) * 128, :],
                                    moe_parts[i][:, sl])
                            nc.gpsimd.collective_compute(
                                "ReduceScatter",
                                bass.mybir.AluOpType.add,
                                replica_groups=groups,
                                ins=[ys[n].opt()],
                                outs=[rss[n].opt()],
                            )
                            gdma.dma_start(moe_r[:, sl], rss[n])

            # ---------------- stage 2: memory read + learned activation ------
            with tc.tile_pool(name="st2", bufs=1) as st2:
                if has_mrb:
                    mrb_row = st2.tile([1, M], f32, tag="mrb")
                    dma.dma_start(mrb_row, mrb_d[:])
                moeT_sb = st2.tile([128, KH * 128], f16, tag="moeT")
                exp_sb = st2.tile([128, M], f32, tag="exp")
                expT_sb = st2.tile([128, 16 * 128], f16, tag="expT")
                s_sb = st2.tile([128, H], f32, tag="s")
                out_sb = st2.tile([128, H], f32, tag="o")
                srec = st2.tile([128, 1], f32, tag="srec")

                with tc.tile_pool(name="pt", bufs=1, space="PSUM") as pt:
                    with tc.tile_pool(name="plg", bufs=1, space="PSUM") as plg:
                        lg = [plg.tile([128, 512], f32, tag="lg", bufs=4,
                                       name=f"lg{n}") for n in range(4)]
                        for ch in range(NH):
                            tp = pt.tile([128, 512], f32, tag="tp", bufs=2,
                                         name=f"tpm{ch}")
                            for j in range(4):
                                hk = ch * 4 + j
                                nc.tensor.transpose(tp[:, j * 128:(j + 1) * 128],
                                                    moe_r[:, hk * 128:(hk + 1) * 128],
                                                    identity)
                            nc.vector.tensor_copy(
                                moeT_sb[:, ch * 512:(ch + 1) * 512], tp)
                            for j in range(4):
                                hk = ch * 4 + j
                                for n in range(4):
                                    nc.tensor.matmul(
                                        lg[n],
                                        moeT_sb[:, hk * 128:(hk + 1) * 128],
                                        mrw_tiles[hk][:, n * 512:(n + 1) * 512],
                                        start=(hk == 0),
                                        stop=(hk == KH - 1) and not has_mrb)
                        if has_mrb:
                            for n in range(4):
                                nc.tensor.matmul(lg[n], ones_row[0:1, 0:1],
                                                 mrb_row[0:1, n * 512:(n + 1) * 512],
                                                 start=False, stop=True)

                        nmx = []
                        for n in range(4):
                            t_ = st2.tile([128, 1], f32, tag=f"nmx{n}", name=f"nmx{n}")
                            nc.vector.reduce_max(t_, lg[n], axis=AX.X, negate=True)
                            nmx.append(t_)
                        t01 = st2.tile([128, 1], f32, tag="t01")
                        nc.vector.tensor_scalar_min(t01, nmx[0], nmx[1])
                        t23 = st2.tile([128, 1], f32, tag="t23")
                        nc.vector.tensor_scalar_min(t23, nmx[2], nmx[3])
                        ngm2 = st2.tile([128, 1], f32, tag="ngm2")
                        nc.vector.tensor_scalar_min(ngm2, t01, t23)
                        ses = []
                        for n in range(4):
                            se_ = st2.tile([128, 1], f32, tag=f"ses{n}", name=f"ses{n}")
                            nc.scalar.activation(exp_sb[:, n * 512:(n + 1) * 512],
                                                 lg[n], AF.Exp, bias=ngm2,
                                                 accum_out=se_)
                            ses.append(se_)
                        s01 = st2.tile([128, 1], f32, tag="s01")
                        nc.vector.tensor_tensor(s01, ses[0], ses[1], ALU.add)
                        s23 = st2.tile([128, 1], f32, tag="s23")
                        nc.vector.tensor_tensor(s23, ses[2], ses[3], ALU.add)
                        stot = st2.tile([128, 1], f32, tag="stot")
                        nc.vector.tensor_tensor(stot, s01, s23, ALU.add)
                        nc.vector.reciprocal(srec, stot)

                    for t in range(4):
                        tp = pt.tile([128, 512], f32, tag="tp", bufs=2, name=f"tpe{t}")
                        for j in range(4):
                            mk = t * 4 + j
                            nc.tensor.transpose(tp[:, j * 128:(j + 1) * 128],
                                                exp_sb[:, mk * 128:(mk + 1) * 128],
                                                identity)
                        nc.vector.tensor_copy(expT_sb[:, t * 512:(t + 1) * 512],
                                              tp)

                with tc.tile_pool(name="prd", bufs=1, space="PSUM") as prd:
                    rd = [prd.tile([128, 512], f32, tag="rd", bufs=4,
                                   name=f"rd{n}") for n in range(4)]
                    for mk in range(16):
                        for n in range(4):
                            nc.tensor.matmul(rd[n],
                                             expT_sb[:, mk * 128:(mk + 1) * 128],
                                             mem_tiles[mk][:, n * 512:(n + 1) * 512],
                                             start=(mk == 0), stop=(mk == 15))
                    # s = moe + read_vec/sum  (deferred softmax normalization);
                    # columns >= h1 have moe == 0 by mask structure
                    for n in range(4):
                        sl = slice(n * 512, (n + 1) * 512)
                        if n < NH:
                            nc.vector.scalar_tensor_tensor(s_sb[:, sl], rd[n],
                                                           srec, moe_r[:, sl],
                                                           ALU.mult, ALU.add)
                        else:
                            nc.vector.tensor_scalar_mul(s_sb[:, sl], rd[n], srec)

                # blended learned activation via diag-matmul accumulation.
                # Mish is not in any HW act table; synthesize
                # mish(s) = s * tanh(relu(s) + ln(1 + exp(-|s|))).
                with tc.tile_pool(name="pac", bufs=1, space="PSUM") as pac, \
                     tc.tile_pool(name="brp", bufs=1) as brp:
                    acc = [pac.tile([128, 512], f32, tag="acc", bufs=4,
                                    name=f"acc{n}") for n in range(4)]
                    n_groups = 7
                    gi = [0]

                    def acc_branch(br_tile, ci):
                        diag = brp.tile([128, 128], f32r, tag="d", bufs=2,
                                        name=f"d{gi[0]}")
                        nc.vector.tensor_scalar_mul(diag, identity,
                                                    coeffs_bc[:, ci:ci + 1])
                        for n in range(4):
                            nc.tensor.matmul(acc[n], diag,
                                             br_tile[:, n * 512:(n + 1) * 512],
                                             start=(gi[0] == 0),
                                             stop=(gi[0] == n_groups - 1))
                        gi[0] += 1

                    # --- nl_exp table phase ---
                    relu_br = brp.tile([128, H], f32r, tag="relu")
                    nc.scalar.activation(relu_br, s_sb, AF.Relu)
                    acc_branch(relu_br, 5)
                    # exp(min(s,0)) branch; the -1 of expm1 is folded into the
                    # final subtraction of c_em below
                    mn = brp.tile([128, H], f32, tag="sc1", bufs=2, name="mn")
                    nc.vector.tensor_scalar_min(mn, s_sb, 0.0)
                    em_br = brp.tile([128, H], f32r, tag="b", bufs=2, name="em")
                    nc.scalar.activation(em_br, mn, AF.Exp)
                    acc_branch(em_br, 6)
                    # softplus(s) = relu(s) + ln(1 + exp(-|s|))
                    abs_s = brp.tile([128, H], f32, tag="sc2", bufs=2, name="ab")
                    nc.scalar.activation(abs_s, s_sb, AF.Abs)
                    enab = brp.tile([128, H], f32, tag="sc1", bufs=2, name="en")
                    nc.scalar.activation(enab, abs_s, AF.Exp, scale=-1.0)
                    ep1 = brp.tile([128, H], f32, tag="sc2", bufs=2, name="e1")
                    nc.vector.tensor_scalar_add(ep1, enab, 1.0)
                    ln1p = brp.tile([128, H], f32, tag="sc1", bufs=2, name="ln")
                    nc.scalar.activation(ln1p, ep1, AF.Ln)
                    sp = brp.tile([128, H], f32, tag="sp")
                    nc.vector.tensor_tensor(sp, ln1p, relu_br.bitcast(f32),
                                            ALU.add)
                    # --- sigmoid table phase (Sigmoid + Tanh) ---
                    sg_br = brp.tile([128, H], f32r, tag="b", bufs=2, name="sg")
                    nc.scalar.activation(sg_br, s_sb, AF.Sigmoid)
                    acc_branch(sg_br, 0)
                    th_br = brp.tile([128, H], f32r, tag="b", bufs=2, name="th")
                    nc.scalar.activation(th_br, s_sb, AF.Tanh)
                    acc_branch(th_br, 1)
                    mt = brp.tile([128, H], f32, tag="sc2", bufs=2, name="mt")
                    nc.scalar.activation(mt, sp, AF.Tanh)
                    mish_br = brp.tile([128, H], f32r, tag="b", bufs=2, name="mi")
                    nc.vector.tensor_tensor(mish_br, mt, s_sb, ALU.mult)
                    acc_branch(mish_br, 4)
                    # silu = s * sigmoid(s), on the vector engine
                    sl_br = brp.tile([128, H], f32r, tag="b", bufs=2, name="sl")
                    nc.vector.tensor_tensor(sl_br, sg_br.bitcast(f32), s_sb,
                                            ALU.mult)
                    acc_branch(sl_br, 2)
                    # --- gelu table phase ---
                    gl_br = brp.tile([128, H], f32r, tag="b", bufs=2, name="gl")
                    nc.scalar.activation(gl_br, s_sb, AF.Gelu)
                    acc_branch(gl_br, 3)
                    assert gi[0] == n_groups
                    for n in range(4):
                        nc.vector.tensor_scalar_sub(out_sb[:, n * 512:(n + 1) * 512],
                                                    acc[n], coeffs_bc[:, 6:7])
                dma.dma_start(out_d[:, :], out_sb)
    nc.finalize()
    return nc


def _get_nc(key=None):
    if key is None:
        key = _LAST_KEY
    if key not in _CACHED_NC:
        _CACHED_NC[key] = _build_program(*key)
    return _CACHED_NC[key]


def _r12(a):
    """Round fp32 to the fp32r grid (11 explicit mantissa bits, RNE)."""
    u = np.ascontiguousarray(a).view(np.uint32)
    u = (u + np.uint32(0x7FF) + ((u >> np.uint32(12)) & np.uint32(1))) \
        & np.uint32(0xFFFFF000)
    return u.view(np.float32)


def kernel(**inputs):
    import os
    from concourse.bass_utils import run_bass_kernel_spmd

    f = lambda a: np.ascontiguousarray(np.asarray(a, dtype=np.float32))
    x = f(inputs["x"])
    gate_w = f(inputs["gate_w"])
    expert_w = f(inputs["expert_w"])
    expert_b = f(inputs["expert_b"])
    conn_w1 = f(inputs["conn_w1"])
    conn_b1 = f(inputs["conn_b1"])
    conn_w2 = f(inputs["conn_w2"])
    conn_b2 = f(inputs["conn_b2"])
    neuron_avg = f(inputs["neuron_avg"])
    neuron_mask = f(inputs["neuron_mask"])
    mem_read_w = f(inputs["mem_read_w"])
    mem_read_b = f(inputs["mem_read_b"])
    memory = f(inputs["memory"])
    act_w = f(inputs["act_w"]).reshape(-1)

    g = int(os.environ.get("MOE_G", "2"))
    EL = E // g
    NB = g

    # host prep: softmax blend weights -> 7 branch coefficients
    p = np.exp(act_w - act_w.max())
    p = p / p.sum()
    coef = np.array([[p[0], p[2], p[4], p[5], p[7],
                      p[1] + p[3] + p[6] * SELU_SCALE,
                      p[1] + p[6] * SELU_SCALE * SELU_ALPHA, 0.0]], np.float32)

    # host conn MLP (batch-independent) -> cmask folded into expert weights
    h1v = np.einsum('eh,ehk->ek', neuron_avg, conn_w1) + conn_b1
    h1v = np.maximum(h1v, 0.0, dtype=np.float32)
    cl = np.einsum('ek,ekh->eh', h1v, conn_w2) + conn_b2
    conn = (1.0 / (1.0 + np.exp(-cl))).astype(np.float32)
    cmask = conn * neuron_mask                                   # [E, H]

    # stage-1 live width: columns past the last nonzero mask column are
    # structurally zero in moe_out, so the program skips them entirely
    nz = np.nonzero(neuron_mask.any(axis=0))[0]
    h1 = int(nz[-1]) + 1 if nz.size else 512
    h1 = min(H, max(512, -(-h1 // 512) * 512))

    wp = (expert_w[:, :, :h1] * cmask[:, None, :h1]).astype(np.float16)
    bp = (expert_b[:, :h1] * cmask[:, :h1]).astype(np.float16)
    has_eb = bool(np.any(bp))
    has_mrb = bool(np.any(mem_read_b))

    xT = np.ascontiguousarray(x.T).astype(np.float16)            # [D, B]
    xT_blk = xT.reshape(8, 128, B).transpose(1, 0, 2)            # [128, 8, B]
    mrw_bf = mem_read_w[:h1].reshape(h1 // 128, 128, M).astype(np.float16)
    mem_bf = memory.reshape(16, 128, H).astype(np.float16)
    mrb = np.ascontiguousarray(mem_read_b.reshape(1, M))

    in_maps = []
    for c in range(NCORES):
        if g > 1:
            bg, eg = c >> 1, c & 1
        else:
            bg, eg = c, 0
        gwr = np.roll(gate_w, -eg * EL, axis=1).astype(np.float16)
        ew_c = wp[eg * EL:(eg + 1) * EL]         # [EL, D, h1]
        m = {
            "xT": np.ascontiguousarray(
                xT_blk[:, :, bg * NB * 128:(bg + 1) * NB * 128]),
            "gw": np.ascontiguousarray(gwr.reshape(8, 128, E).transpose(1, 0, 2)),
            "ew": np.ascontiguousarray(
                ew_c.reshape(EL, 8, 128, h1).transpose(0, 2, 1, 3)),
            "mrw": mrw_bf,
            "mem": mem_bf,
            "coef": coef,
        }
        if has_eb:
            m["eb"] = np.ascontiguousarray(
                bp[eg * EL:(eg + 1) * EL].reshape(EL, 1, h1))
        if has_mrb:
            m["mrb"] = mrb
        in_maps.append(m)

    key = (h1, g, has_eb, has_mrb)
    global _LAST_IN_MAPS, _LAST_KEY
    _LAST_IN_MAPS = in_maps
    _LAST_KEY = key
    nc = _get_nc(key)
    results = run_bass_kernel_spmd(nc, in_maps, list(range(NCORES))).results
    out = np.concatenate(
        [np.asarray(results[c]["out"], dtype=np.float32) for c in range(NCORES)],
        axis=0)
    return out


# revision 10
# speedup vs baseline: 2.1326x; 1.2056x over previous
"""Hybrid expert/batch-parallel Trainium2 kernel for PlasticityModelMoE.

Sharding (g=2 expert-parallel x b=4 batch-parallel): core c = (batch group
bg=c>>1, expert group eg=c&1). Each core computes its 4 experts' gate-weighted
sum for its 256 batch rows, then ONE pairwise ReduceScatter(add) over
[[0,1],[2,3],[4,5],[6,7]] (512KB) leaves core c with batch rows
[128c, 128c+128). Stage 2 (episodic-memory attention + blended learned
activation) runs on those 128 rows with replicated mem_read_w / memory in
bf16. Host concatenates the 8 [128, 2048] outputs.

Host-side folding: the DynamicConnectivity MLP depends only on neuron_avg
(batch-independent), so cmask = sigmoid(conn)*neuron_mask is computed on the
host and folded into expert_w columns (relu(z*m) == m*relu(z) for m>=0);
device stage 1 is then just gate softmax + z matmuls + relu/gate-scale
accumulation. Columns past the last nonzero mask column are structurally zero
in moe_out, so only h1 columns are computed (and only h1 rows of mem_read_w
are loaded).

DMA rings: sync + vector stream the folded expert weights (half-expert
tiles); scalar(ACT) prefetches mem_read_w then memory (bf16); gpsimd carries
xT/gw/coef, the RS traffic, and the output.
"""

import numpy as np

B, D, H, E, M = 1024, 1024, 2048, 8, 2048
NCORES = 8
SELU_SCALE = 1.0507009873554805
SELU_ALPHA = 1.6732632423543772

_CACHED_NC = {}
_LAST_KEY = None
_LAST_IN_MAPS = None


def _build_program(h1, g, has_eb, has_mrb):
    import concourse.bass as bass
    from concourse import bacc, mybir, tile
    from concourse.masks import make_identity

    f32 = mybir.dt.float32
    f32r = mybir.dt.float32r
    f16 = mybir.dt.float16
    EL = E // g          # experts per core
    NB = g               # 128-row batch blocks per core
    NH = h1 // 512       # stage-1 column chunks
    KH = h1 // 128       # K blocks for the attention logits
    AF = mybir.ActivationFunctionType
    ALU = mybir.AluOpType
    AX = mybir.AxisListType

    nc = bacc.Bacc(None, target_bir_lowering=False, debug=False)

    xT_d = nc.dram_tensor("xT", [128, 8, NB * 128], f16, kind="ExternalInput")
    gw_d = nc.dram_tensor("gw", [128, 8, E], f16, kind="ExternalInput")
    ew_d = nc.dram_tensor("ew", [EL, 128, 8, h1], f16, kind="ExternalInput")
    if has_eb:
        eb_d = nc.dram_tensor("eb", [EL, 1, h1], f16, kind="ExternalInput")
    mrw_d = nc.dram_tensor("mrw", [KH, 128, M], f16, kind="ExternalInput")
    if has_mrb:
        mrb_d = nc.dram_tensor("mrb", [1, M], f32, kind="ExternalInput")
    mem_d = nc.dram_tensor("mem", [16, 128, H], f16, kind="ExternalInput")
    cf_d = nc.dram_tensor("coef", [1, 8], f32, kind="ExternalInput")
    out_d = nc.dram_tensor("out", [128, H], f32, kind="ExternalOutput")

    # sync carries the early-critical + most bulk traffic (it has no compute
    # duties so its in-order dma_start queue can block freely); scalar gets a
    # short queue so epilogue ACTs are never blocked behind DMA; gpsimd (slow
    # SW DGE) gets RS traffic, the output, and residual mem tiles.
    dma = nc.default_dma_engine   # SP hwdge ring
    adma = nc.scalar              # ACT hwdge ring
    gdma = nc.gpsimd              # gpsimd SW ring

    with tile.TileContext(nc) as tc:
        with tc.tile_pool(name="consts", bufs=1) as consts, \
             tc.tile_pool(name="dramp", bufs=1, space="DRAM") as dramp, \
             tc.tile_pool(name="mrwp", bufs=KH) as mrwp, \
             tc.tile_pool(name="memp", bufs=16) as memp:

            identity = consts.tile([128, 128], f32, tag="idn")
            make_identity(nc, identity)
            ones_row = consts.tile([1, 128], f32, tag="ones")
            nc.vector.memset(ones_row, 1.0)
            coef_row = consts.tile([1, 8], f32, tag="coef")
            dma.dma_start(coef_row, cf_d[:])
            coeffs_bc = consts.tile([128, 8], f32, tag="cfb")
            moe_r = consts.tile([128, h1], f32, tag="moer")


            if g > 1:
                ys = [dramp.tile([NB * 128, 512], f32, tag=f"y{n}", name=f"y{n}")
                      for n in range(NH)]
                rss = [dramp.tile([128, 512], f32, tag=f"rs{n}", name=f"rs{n}")
                       for n in range(NH)]
                groups = [[2 * k, 2 * k + 1] for k in range(4)]

            # ---------------- stage 1: hybrid-parallel MoE ----------------
            with tc.tile_pool(name="w1", bufs=1) as w1, \
                 tc.tile_pool(name="ewp", bufs=(8 if g > 1 else 4)) as ewp, \
                 tc.tile_pool(name="pb", bufs=1, space="PSUM") as pb:
                xT_sb = w1.tile([128, 8, NB * 128], f16, tag="xT")
                dma.dma_start(xT_sb, xT_d[:])
                gw_sb = w1.tile([128, 8, E], f16, tag="gw")
                adma.dma_start(gw_sb, gw_d[:])
                if has_eb:
                    eb_rows = w1.tile([EL, 1, h1], f16, tag="eb")
                    dma.dma_start(eb_rows, eb_d[:])
                    ones_f16 = w1.tile([1, 128], f16, tag="o16")
                    nc.vector.memset(ones_f16, 1.0)

                # gate softmax FIRST in scalar program order so its Exp
                # is never stuck behind bulk dma_starts on the ACT queue
                gcols = []
                for i in range(NB):
                    bs = slice(i * 128, (i + 1) * 128)
                    gate_ps = pb.tile([128, E], f32, tag="g", bufs=2, name=f"g{i}")
                    for k in range(8):
                        nc.tensor.matmul(gate_ps, xT_sb[:, k, bs], gw_sb[:, k, :],
                                         start=(k == 0), stop=(k == 7))
                    ngm = w1.tile([128, 1], f32, tag="ngm", bufs=2, name=f"ngm{i}")
                    nc.vector.reduce_max(ngm, gate_ps, axis=AX.X, negate=True)
                    eg_t = w1.tile([128, E], f32, tag="eg", bufs=2, name=f"eg{i}")
                    sume = w1.tile([128, 1], f32, tag="se", bufs=2, name=f"se{i}")
                    nc.scalar.activation(eg_t, gate_ps, AF.Exp, bias=ngm,
                                         accum_out=sume)
                    rec = w1.tile([128, 1], f32, tag="rec", bufs=2, name=f"rec{i}")
                    nc.vector.reciprocal(rec, sume)
                    cols = []
                    for j in range(EL):
                        gcol = w1.tile([128, 1], f32, tag=f"gc{i}_{j}",
                                       name=f"gc{i}_{j}")
                        nc.vector.tensor_scalar_mul(gcol, eg_t[:, j:j + 1], rec)
                        cols.append(gcol)
                    gcols.append(cols)

                # expert weight stream: half-expert tiles on two rings
                ew_tiles = []
                for e in range(EL):
                    t0 = ewp.tile([128, 4, h1], f16, tag="ew", name=f"ew{e}h0")
                    dma.dma_start(t0, ew_d[e, :, 0:4, :])
                    t1 = ewp.tile([128, 4, h1], f16, tag="ew", name=f"ew{e}h1")
                    adma.dma_start(t1, ew_d[e, :, 4:8, :])
                    ew_tiles.append((t0, t1))

                # stage-2 weights queued behind the expert stream, split so
                # each ring drains in time for its first consumer
                mrw_tiles = []
                for hk in range(KH):
                    t_ = mrwp.tile([128, M], f16, tag="w", name=f"mrw{hk}")
                    (dma if hk < 4 else adma).dma_start(t_, mrw_d[hk])
                    mrw_tiles.append(t_)
                mem_tiles = []
                for mk in range(16):
                    t_ = memp.tile([128, H], f16, tag="m", name=f"mem{mk}")
                    if g == 1:
                        eng = dma if mk < 5 else (adma if mk < 10 else gdma)
                    else:
                        eng = dma if mk < 6 else (adma if mk < 12 else gdma)
                    eng.dma_start(t_, mem_d[mk])
                    mem_tiles.append(t_)

                # broadcast activation-blend coefficients to 128 partitions
                cf_ps = pb.tile([128, 8], f32, tag="cf")
                nc.tensor.matmul(cf_ps, ones_row, coef_row, start=True, stop=True)
                nc.vector.tensor_copy(coeffs_bc, cf_ps)

                moe_parts = [moe_r] if g == 1 else [
                    w1.tile([128, h1], f32, tag=f"mp{i}", name=f"mp{i}")
                    for i in range(NB)]

                # g>1: chunk-major (all ew resident) so the RS for chunk n
                # fires mid-compute; g==1: expert-major streaming (no RS, and
                # 16 resident half-tiles would not fit SBUF)
                loop_order = ([(n, e) for n in range(NH) for e in range(EL)]
                              if g > 1 else
                              [(n, e) for e in range(EL) for n in range(NH)])
                for n, e in loop_order:
                        sl = slice(n * 512, (n + 1) * 512)
                        t0, t1 = ew_tiles[e]
                        for i in range(NB):
                            bs = slice(i * 128, (i + 1) * 128)
                            z_ps = pb.tile([128, 512], f32, tag="z", bufs=4,
                                           name=f"z{e}_{n}_{i}")
                            for k in range(4):
                                nc.tensor.matmul(z_ps, xT_sb[:, k, bs],
                                                 t0[:, k, sl],
                                                 start=(k == 0), stop=False)
                            for k in range(4):
                                last = (k == 3) and not has_eb
                                nc.tensor.matmul(z_ps, xT_sb[:, 4 + k, bs],
                                                 t1[:, k, sl],
                                                 start=False, stop=last)
                            if has_eb:
                                nc.tensor.matmul(z_ps, ones_f16[0:1, 0:1],
                                                 eb_rows[e, 0:1, sl],
                                                 start=False, stop=True)
                            # relu + gate-scale + accumulate, all on DVE so
                            # the ACT queue never gates PSUM recycling
                            t_ = w1.tile([128, 512], f32, tag="acc", bufs=3,
                                         name=f"a{e}_{n}_{i}")
                            nc.vector.tensor_scalar_max(t_, z_ps, 0.0)
                            if e == 0:
                                nc.vector.tensor_scalar_mul(
                                    moe_parts[i][:, sl], t_, gcols[i][e])
                            else:
                                nc.vector.scalar_tensor_tensor(
                                    moe_parts[i][:, sl], t_, gcols[i][e],
                                    moe_parts[i][:, sl], ALU.mult, ALU.add)

                        # pairwise ReduceScatter as soon as chunk n done
                        if g > 1 and e == EL - 1:
                            for i in range(NB):
                                gdma.dma_start(
                                    ys[n][i * 128:(i + 1) * 128, :],
                                    moe_parts[i][:, sl])
                            nc.gpsimd.collective_compute(
                                "ReduceScatter",
                                bass.mybir.AluOpType.add,
                                replica_groups=groups,
                                ins=[ys[n].opt()],
                                outs=[rss[n].opt()],
                            )
                            gdma.dma_start(moe_r[:, sl], rss[n])

            # ---------------- stage 2: memory read + learned activation ------
            with tc.tile_pool(name="st2", bufs=1) as st2:
                if has_mrb:
                    mrb_row = st2.tile([1, M], f32, tag="mrb")
                    dma.dma_start(mrb_row, mrb_d[:])
                moeT_sb = st2.tile([128, KH * 128], f16, tag="moeT")
                exp_sb = st2.tile([128, M], f32, tag="exp")
                expT_sb = st2.tile([128, 16 * 128], f16, tag="expT")
                s_sb = st2.tile([128, H], f32, tag="s")
                out_sb = st2.tile([128, H], f32, tag="o")
                srec = st2.tile([128, 1], f32, tag="srec")

                with tc.tile_pool(name="pt", bufs=1, space="PSUM") as pt:
                    with tc.tile_pool(name="plg", bufs=1, space="PSUM") as plg:
                        lg = [plg.tile([128, 512], f32, tag="lg", bufs=4,
                                       name=f"lg{n}") for n in range(4)]
                        for ch in range(NH):
                            tp = pt.tile([128, 512], f32, tag="tp", bufs=2,
                                         name=f"tpm{ch}")
                            for j in range(4):
                                hk = ch * 4 + j
                                nc.tensor.transpose(tp[:, j * 128:(j + 1) * 128],
                                                    moe_r[:, hk * 128:(hk + 1) * 128],
                                                    identity)
                            nc.vector.tensor_copy(
                                moeT_sb[:, ch * 512:(ch + 1) * 512], tp)
                            for j in range(4):
                                hk = ch * 4 + j
                                for n in range(4):
                                    nc.tensor.matmul(
                                        lg[n],
                                        moeT_sb[:, hk * 128:(hk + 1) * 128],
                                        mrw_tiles[hk][:, n * 512:(n + 1) * 512],
                                        start=(hk == 0),
                                        stop=(hk == KH - 1) and not has_mrb)
                        if has_mrb:
                            for n in range(4):
                                nc.tensor.matmul(lg[n], ones_row[0:1, 0:1],
                                                 mrb_row[0:1, n * 512:(n + 1) * 512],
                                                 start=False, stop=True)

                        nmx = []
                        for n in range(4):
                            t_ = st2.tile([128, 1], f32, tag=f"nmx{n}", name=f"nmx{n}")
                            nc.vector.reduce_max(t_, lg[n], axis=AX.X, negate=True)
                            nmx.append(t_)
                        t01 = st2.tile([128, 1], f32, tag="t01")
                        nc.vector.tensor_scalar_min(t01, nmx[0], nmx[1])
                        t23 = st2.tile([128, 1], f32, tag="t23")
                        nc.vector.tensor_scalar_min(t23, nmx[2], nmx[3])
                        ngm2 = st2.tile([128, 1], f32, tag="ngm2")
                        nc.vector.tensor_scalar_min(ngm2, t01, t23)
                        ses = []
                        for n in range(4):
                            se_ = st2.tile([128, 1], f32, tag=f"ses{n}", name=f"ses{n}")
                            nc.scalar.activation(exp_sb[:, n * 512:(n + 1) * 512],
                                                 lg[n], AF.Exp, bias=ngm2,
                                                 accum_out=se_)
                            ses.append(se_)
                        s01 = st2.tile([128, 1], f32, tag="s01")
                        nc.vector.tensor_tensor(s01, ses[0], ses[1], ALU.add)
                        s23 = st2.tile([128, 1], f32, tag="s23")
                        nc.vector.tensor_tensor(s23, ses[2], ses[3], ALU.add)
                        stot = st2.tile([128, 1], f32, tag="stot")
                        nc.vector.tensor_tensor(stot, s01, s23, ALU.add)
                        nc.vector.reciprocal(srec, stot)

                    for t in range(4):
                        tp = pt.tile([128, 512], f32, tag="tp", bufs=2, name=f"tpe{t}")
                        for j in range(4):
                            mk = t * 4 + j
                            nc.tensor.transpose(tp[:, j * 128:(j + 1) * 128],
                                                exp_sb[:, mk * 128:(mk + 1) * 128],
                                                identity)
                        nc.vector.tensor_copy(expT_sb[:, t * 512:(t + 1) * 512],
                                              tp)

                with tc.tile_pool(name="prd", bufs=1, space="PSUM") as prd:
                    rd = [prd.tile([128, 512], f32, tag="rd", bufs=4,
                                   name=f"rd{n}") for n in range(4)]
                    for mk in range(16):
                        for n in range(4):
                            nc.tensor.matmul(rd[n],
                                             expT_sb[:, mk * 128:(mk + 1) * 128],
                                             mem_tiles[mk][:, n * 512:(n + 1) * 512],
                                             start=(mk == 0), stop=(mk == 15))
                    # s = moe + read_vec/sum  (deferred softmax normalization);
                    # columns >= h1 have moe == 0 by mask structure
                    for n in range(4):
                        sl = slice(n * 512, (n + 1) * 512)
                        if n < NH:
                            nc.vector.scalar_tensor_tensor(s_sb[:, sl], rd[n],
                                                           srec, moe_r[:, sl],
                                                           ALU.mult, ALU.add)
                        else:
                            nc.vector.tensor_scalar_mul(s_sb[:, sl], rd[n], srec)

                # blended learned activation via diag-matmul accumulation.
                # Mish is not in any HW act table; synthesize
                # mish(s) = s * tanh(relu(s) + ln(1 + exp(-|s|))).
                with tc.tile_pool(name="pac", bufs=1, space="PSUM") as pac, \
                     tc.tile_pool(name="brp", bufs=1) as brp:
                    acc = [pac.tile([128, 512], f32, tag="acc", bufs=4,
                                    name=f"acc{n}") for n in range(4)]
                    n_groups = 7
                    gi = [0]

                    def acc_branch(br_tile, ci):
                        diag = brp.tile([128, 128], f32r, tag="d", bufs=2,
                                        name=f"d{gi[0]}")
                        nc.vector.tensor_scalar_mul(diag, identity,
                                                    coeffs_bc[:, ci:ci + 1])
                        for n in range(4):
                            nc.tensor.matmul(acc[n], diag,
                                             br_tile[:, n * 512:(n + 1) * 512],
                                             start=(gi[0] == 0),
                                             stop=(gi[0] == n_groups - 1))
                        gi[0] += 1

                    # --- nl_exp table phase ---
                    relu_br = brp.tile([128, H], f32r, tag="relu")
                    nc.scalar.activation(relu_br, s_sb, AF.Relu)
                    acc_branch(relu_br, 5)
                    # exp(min(s,0)) branch; the -1 of expm1 is folded into the
                    # final subtraction of c_em below
                    mn = brp.tile([128, H], f32, tag="sc1", bufs=2, name="mn")
                    nc.vector.tensor_scalar_min(mn, s_sb, 0.0)
                    em_br = brp.tile([128, H], f32r, tag="b", bufs=2, name="em")
                    nc.scalar.activation(em_br, mn, AF.Exp)
                    acc_branch(em_br, 6)
                    # softplus(s) = relu(s) + ln(1 + exp(-|s|))
                    abs_s = brp.tile([128, H], f32, tag="sc2", bufs=2, name="ab")
                    nc.scalar.activation(abs_s, s_sb, AF.Abs)
                    enab = brp.tile([128, H], f32, tag="sc1", bufs=2, name="en")
                    nc.scalar.activation(enab, abs_s, AF.Exp, scale=-1.0)
                    ep1 = brp.tile([128, H], f32, tag="sc2", bufs=2, name="e1")
                    nc.vector.tensor_scalar_add(ep1, enab, 1.0)
                    ln1p = brp.tile([128, H], f32, tag="sc1", bufs=2, name="ln")
                    nc.scalar.activation(ln1p, ep1, AF.Ln)
                    sp = brp.tile([128, H], f32, tag="sp")
                    nc.vector.tensor_tensor(sp, ln1p, relu_br.bitcast(f32),
                                            ALU.add)
                    # --- sigmoid table phase (Sigmoid + Tanh) ---
                    sg_br = brp.tile([128, H], f32r, tag="b", bufs=2, name="sg")
                    nc.scalar.activation(sg_br, s_sb, AF.Sigmoid)
                    acc_branch(sg_br, 0)
                    th_br = brp.tile([128, H], f32r, tag="b", bufs=2, name="th")
                    nc.scalar.activation(th_br, s_sb, AF.Tanh)
                    acc_branch(th_br, 1)
                    mt = brp.tile([128, H], f32, tag="sc2", bufs=2, name="mt")
                    nc.scalar.activation(mt, sp, AF.Tanh)
                    mish_br = brp.tile([128, H], f32r, tag="b", bufs=2, name="mi")
                    nc.vector.tensor_tensor(mish_br, mt, s_sb, ALU.mult)
                    acc_branch(mish_br, 4)
                    # silu = s * sigmoid(s), on the vector engine
                    sl_br = brp.tile([128, H], f32r, tag="b", bufs=2, name="sl")
                    nc.vector.tensor_tensor(sl_br, sg_br.bitcast(f32), s_sb,
                                            ALU.mult)
                    acc_branch(sl_br, 2)
                    # --- gelu table phase ---
                    gl_br = brp.tile([128, H], f32r, tag="b", bufs=2, name="gl")
                    nc.scalar.activation(gl_br, s_sb, AF.Gelu)
                    acc_branch(gl_br, 3)
                    assert gi[0] == n_groups
                    for n in range(4):
                        nc.vector.tensor_scalar_sub(out_sb[:, n * 512:(n + 1) * 512],
                                                    acc[n], coeffs_bc[:, 6:7])
                dma.dma_start(out_d[:, :], out_sb)
    nc.finalize()
    return nc


def _get_nc(key=None):
    if key is None:
        key = _LAST_KEY
    if key not in _CACHED_NC:
        _CACHED_NC[key] = _build_program(*key)
    return _CACHED_NC[key]


def _r12(a):
    """Round fp32 to the fp32r grid (11 explicit mantissa bits, RNE)."""
    u = np.ascontiguousarray(a).view(np.uint32)
    u = (u + np.uint32(0x7FF) + ((u >> np.uint32(12)) & np.uint32(1))) \
        & np.uint32(0xFFFFF000)
    return u.view(np.float32)


def kernel(**inputs):
    import os
    from concourse.bass_utils import run_bass_kernel_spmd

    f = lambda a: np.ascontiguousarray(np.asarray(a, dtype=np.float32))
    x = f(inputs["x"])
    gate_w = f(inputs["gate_w"])
    expert_w = f(inputs["expert_w"])
    expert_b = f(inputs["expert_b"])
    conn_w1 = f(inputs["conn_w1"])
    conn_b1 = f(inputs["conn_b1"])
    conn_w2 = f(inputs["conn_w2"])
    conn_b2 = f(inputs["conn_b2"])
    neuron_avg = f(inputs["neuron_avg"])
    neuron_mask = f(inputs["neuron_mask"])
    mem_read_w = f(inputs["mem_read_w"])
    mem_read_b = f(inputs["mem_read_b"])
    memory = f(inputs["memory"])
    act_w = f(inputs["act_w"]).reshape(-1)

    g = int(os.environ.get("MOE_G", "1"))
    EL = E // g
    NB = g

    # host prep: softmax blend weights -> 7 branch coefficients
    p = np.exp(act_w - act_w.max())
    p = p / p.sum()
    coef = np.array([[p[0], p[2], p[4], p[5], p[7],
                      p[1] + p[3] + p[6] * SELU_SCALE,
                      p[1] + p[6] * SELU_SCALE * SELU_ALPHA, 0.0]], np.float32)

    # host conn MLP (batch-independent) -> cmask folded into expert weights
    h1v = np.einsum('eh,ehk->ek', neuron_avg, conn_w1) + conn_b1
    h1v = np.maximum(h1v, 0.0, dtype=np.float32)
    cl = np.einsum('ek,ekh->eh', h1v, conn_w2) + conn_b2
    conn = (1.0 / (1.0 + np.exp(-cl))).astype(np.float32)
    cmask = conn * neuron_mask                                   # [E, H]

    # stage-1 live width: columns past the last nonzero mask column are
    # structurally zero in moe_out, so the program skips them entirely
    nz = np.nonzero(neuron_mask.any(axis=0))[0]
    h1 = int(nz[-1]) + 1 if nz.size else 512
    h1 = min(H, max(512, -(-h1 // 512) * 512))

    wp = (expert_w[:, :, :h1] * cmask[:, None, :h1]).astype(np.float16)
    bp = (expert_b[:, :h1] * cmask[:, :h1]).astype(np.float16)
    has_eb = bool(np.any(bp))
    has_mrb = bool(np.any(mem_read_b))

    xT = np.ascontiguousarray(x.T).astype(np.float16)            # [D, B]
    xT_blk = xT.reshape(8, 128, B).transpose(1, 0, 2)            # [128, 8, B]
    mrw_bf = mem_read_w[:h1].reshape(h1 // 128, 128, M).astype(np.float16)
    mem_bf = memory.reshape(16, 128, H).astype(np.float16)
    mrb = np.ascontiguousarray(mem_read_b.reshape(1, M))

    in_maps = []
    for c in range(NCORES):
        if g > 1:
            bg, eg = c >> 1, c & 1
        else:
            bg, eg = c, 0
        gwr = np.roll(gate_w, -eg * EL, axis=1).astype(np.float16)
        ew_c = wp[eg * EL:(eg + 1) * EL]         # [EL, D, h1]
        m = {
            "xT": np.ascontiguousarray(
                xT_blk[:, :, bg * NB * 128:(bg + 1) * NB * 128]),
            "gw": np.ascontiguousarray(gwr.reshape(8, 128, E).transpose(1, 0, 2)),
            "ew": np.ascontiguousarray(
                ew_c.reshape(EL, 8, 128, h1).transpose(0, 2, 1, 3)),
            "mrw": mrw_bf,
            "mem": mem_bf,
            "coef": coef,
        }
        if has_eb:
            m["eb"] = np.ascontiguousarray(
                bp[eg * EL:(eg + 1) * EL].reshape(EL, 1, h1))
        if has_mrb:
            m["mrb"] = mrb
        in_maps.append(m)

    key = (h1, g, has_eb, has_mrb)
    global _LAST_IN_MAPS, _LAST_KEY
    _LAST_IN_MAPS = in_maps
    _LAST_KEY = key
    nc = _get_nc(key)
    results = run_bass_kernel_spmd(nc, in_maps, list(range(NCORES))).results
    out = np.concatenate(
        [np.asarray(results[c]["out"], dtype=np.float32) for c in range(NCORES)],
        axis=0)
    return out


# revision 11
# speedup vs baseline: 2.3033x; 1.0801x over previous
"""Hybrid expert/batch-parallel Trainium2 kernel for PlasticityModelMoE.

Sharding (g=2 expert-parallel x b=4 batch-parallel): core c = (batch group
bg=c>>1, expert group eg=c&1). Each core computes its 4 experts' gate-weighted
sum for its 256 batch rows, then ONE pairwise ReduceScatter(add) over
[[0,1],[2,3],[4,5],[6,7]] (512KB) leaves core c with batch rows
[128c, 128c+128). Stage 2 (episodic-memory attention + blended learned
activation) runs on those 128 rows with replicated mem_read_w / memory in
bf16. Host concatenates the 8 [128, 2048] outputs.

Host-side folding: the DynamicConnectivity MLP depends only on neuron_avg
(batch-independent), so cmask = sigmoid(conn)*neuron_mask is computed on the
host and folded into expert_w columns (relu(z*m) == m*relu(z) for m>=0);
device stage 1 is then just gate softmax + z matmuls + relu/gate-scale
accumulation. Columns past the last nonzero mask column are structurally zero
in moe_out, so only h1 columns are computed (and only h1 rows of mem_read_w
are loaded).

DMA rings: sync + vector stream the folded expert weights (half-expert
tiles); scalar(ACT) prefetches mem_read_w then memory (bf16); gpsimd carries
xT/gw/coef, the RS traffic, and the output.
"""

import numpy as np

B, D, H, E, M = 1024, 1024, 2048, 8, 2048
NCORES = 8
SELU_SCALE = 1.0507009873554805
SELU_ALPHA = 1.6732632423543772

_CACHED_NC = {}
_LAST_KEY = None
_LAST_IN_MAPS = None


def _build_program(h1, g, has_eb, has_mrb):
    import concourse.bass as bass
    from concourse import bacc, mybir, tile
    from concourse.masks import make_identity

    f32 = mybir.dt.float32
    f32r = mybir.dt.float32r
    f16 = mybir.dt.float16
    EL = E // g          # experts per core
    NB = g               # 128-row batch blocks per core
    NH = h1 // 512       # stage-1 column chunks
    KH = h1 // 128       # K blocks for the attention logits
    AF = mybir.ActivationFunctionType
    ALU = mybir.AluOpType
    AX = mybir.AxisListType

    nc = bacc.Bacc(None, target_bir_lowering=False, debug=False)

    xT_d = nc.dram_tensor("xT", [128, 8, NB * 128], f16, kind="ExternalInput")
    gw_d = nc.dram_tensor("gw", [128, 8, E], f16, kind="ExternalInput")
    ew_d = nc.dram_tensor("ew", [EL, 128, 8, h1], f16, kind="ExternalInput")
    if has_eb:
        eb_d = nc.dram_tensor("eb", [EL, 1, h1], f16, kind="ExternalInput")
    mrw_d = nc.dram_tensor("mrw", [KH, 128, M], f16, kind="ExternalInput")
    if has_mrb:
        mrb_d = nc.dram_tensor("mrb", [1, M], f32, kind="ExternalInput")
    mem_d = nc.dram_tensor("mem", [16, 128, H], f16, kind="ExternalInput")
    cf_d = nc.dram_tensor("coef", [1, 8], f32, kind="ExternalInput")
    out_d = nc.dram_tensor("out", [128, H], f32, kind="ExternalOutput")

    # sync carries the early-critical + most bulk traffic (it has no compute
    # duties so its in-order dma_start queue can block freely); scalar gets a
    # short queue so epilogue ACTs are never blocked behind DMA; gpsimd (slow
    # SW DGE) gets RS traffic, the output, and residual mem tiles.
    dma = nc.default_dma_engine   # SP hwdge ring
    adma = nc.scalar              # ACT hwdge ring
    gdma = nc.gpsimd              # gpsimd SW ring

    with tile.TileContext(nc) as tc:
        with tc.tile_pool(name="consts", bufs=1) as consts, \
             tc.tile_pool(name="dramp", bufs=1, space="DRAM") as dramp, \
             tc.tile_pool(name="mrwp", bufs=KH) as mrwp, \
             tc.tile_pool(name="memp", bufs=16) as memp:

            identity = consts.tile([128, 128], f32, tag="idn")
            make_identity(nc, identity)
            ones_row = consts.tile([1, 128], f32, tag="ones")
            nc.vector.memset(ones_row, 1.0)
            coef_row = consts.tile([1, 8], f32, tag="coef")
            dma.dma_start(coef_row, cf_d[:])
            coeffs_bc = consts.tile([128, 8], f32, tag="cfb")
            moe_r = consts.tile([128, h1], f32, tag="moer")


            if g > 1:
                ys = [dramp.tile([NB * 128, 512], f32, tag=f"y{n}", name=f"y{n}")
                      for n in range(NH)]
                rss = [dramp.tile([128, 512], f32, tag=f"rs{n}", name=f"rs{n}")
                       for n in range(NH)]
                groups = [[2 * k, 2 * k + 1] for k in range(4)]

            # ---------------- stage 1: hybrid-parallel MoE ----------------
            with tc.tile_pool(name="w1", bufs=1) as w1, \
                 tc.tile_pool(name="ewp", bufs=(8 if g > 1 else 4)) as ewp, \
                 tc.tile_pool(name="pb", bufs=1, space="PSUM") as pb:
                xT_sb = w1.tile([128, 8, NB * 128], f16, tag="xT")
                dma.dma_start(xT_sb, xT_d[:])
                gw_sb = w1.tile([128, 8, E], f16, tag="gw")
                adma.dma_start(gw_sb, gw_d[:])
                if has_eb:
                    eb_rows = w1.tile([EL, 1, h1], f16, tag="eb")
                    dma.dma_start(eb_rows, eb_d[:])
                    ones_f16 = w1.tile([1, 128], f16, tag="o16")
                    nc.vector.memset(ones_f16, 1.0)

                # gate softmax FIRST in scalar program order so its Exp
                # is never stuck behind bulk dma_starts on the ACT queue
                gcols = []
                for i in range(NB):
                    bs = slice(i * 128, (i + 1) * 128)
                    gate_ps = pb.tile([128, E], f32, tag="g", bufs=2, name=f"g{i}")
                    for k in range(8):
                        nc.tensor.matmul(gate_ps, xT_sb[:, k, bs], gw_sb[:, k, :],
                                         start=(k == 0), stop=(k == 7))
                    ngm = w1.tile([128, 1], f32, tag="ngm", bufs=2, name=f"ngm{i}")
                    nc.vector.reduce_max(ngm, gate_ps, axis=AX.X, negate=True)
                    eg_t = w1.tile([128, E], f32, tag="eg", bufs=2, name=f"eg{i}")
                    sume = w1.tile([128, 1], f32, tag="se", bufs=2, name=f"se{i}")
                    nc.scalar.activation(eg_t, gate_ps, AF.Exp, bias=ngm,
                                         accum_out=sume)
                    rec = w1.tile([128, 1], f32, tag="rec", bufs=2, name=f"rec{i}")
                    nc.vector.reciprocal(rec, sume)
                    cols = []
                    for j in range(EL):
                        gcol = w1.tile([128, 1], f32, tag=f"gc{i}_{j}",
                                       name=f"gc{i}_{j}")
                        nc.vector.tensor_scalar_mul(gcol, eg_t[:, j:j + 1], rec)
                        cols.append(gcol)
                    gcols.append(cols)

                # expert weight stream: half-expert tiles on two rings
                ew_tiles = []
                for e in range(EL):
                    t0 = ewp.tile([128, 4, h1], f16, tag="ew", name=f"ew{e}h0")
                    dma.dma_start(t0, ew_d[e, :, 0:4, :])
                    t1 = ewp.tile([128, 4, h1], f16, tag="ew", name=f"ew{e}h1")
                    adma.dma_start(t1, ew_d[e, :, 4:8, :])
                    ew_tiles.append((t0, t1))

                # stage-2 weights queued behind the expert stream, split so
                # each ring drains in time for its first consumer
                mrw_tiles = []
                for hk in range(KH):
                    t_ = mrwp.tile([128, M], f16, tag="w", name=f"mrw{hk}")
                    (dma if hk < 4 else adma).dma_start(t_, mrw_d[hk])
                    mrw_tiles.append(t_)
                mem_tiles = []
                for mk in range(16):
                    t_ = memp.tile([128, H], f16, tag="m", name=f"mem{mk}")
                    if g == 1:
                        eng = dma if mk < 5 else (adma if mk < 10 else gdma)
                    else:
                        eng = dma if mk < 6 else (adma if mk < 12 else gdma)
                    eng.dma_start(t_, mem_d[mk])
                    mem_tiles.append(t_)

                # broadcast activation-blend coefficients to 128 partitions
                cf_ps = pb.tile([128, 8], f32, tag="cf")
                nc.tensor.matmul(cf_ps, ones_row, coef_row, start=True, stop=True)
                nc.vector.tensor_copy(coeffs_bc, cf_ps)

                moe_parts = [moe_r] if g == 1 else [
                    w1.tile([128, h1], f32, tag=f"mp{i}", name=f"mp{i}")
                    for i in range(NB)]

                # g>1: chunk-major (all ew resident) so the RS for chunk n
                # fires mid-compute; g==1: expert-major streaming (no RS, and
                # 16 resident half-tiles would not fit SBUF)
                loop_order = ([(n, e) for n in range(NH) for e in range(EL)]
                              if g > 1 else
                              [(n, e) for e in range(EL) for n in range(NH)])
                for n, e in loop_order:
                        sl = slice(n * 512, (n + 1) * 512)
                        t0, t1 = ew_tiles[e]
                        for i in range(NB):
                            bs = slice(i * 128, (i + 1) * 128)
                            z_ps = pb.tile([128, 512], f32, tag="z", bufs=4,
                                           name=f"z{e}_{n}_{i}")
                            for k in range(4):
                                nc.tensor.matmul(z_ps, xT_sb[:, k, bs],
                                                 t0[:, k, sl],
                                                 start=(k == 0), stop=False)
                            for k in range(4):
                                last = (k == 3) and not has_eb
                                nc.tensor.matmul(z_ps, xT_sb[:, 4 + k, bs],
                                                 t1[:, k, sl],
                                                 start=False, stop=last)
                            if has_eb:
                                nc.tensor.matmul(z_ps, ones_f16[0:1, 0:1],
                                                 eb_rows[e, 0:1, sl],
                                                 start=False, stop=True)
                            # relu + gate-scale + accumulate, all on DVE so
                            # the ACT queue never gates PSUM recycling
                            t_ = w1.tile([128, 512], f32, tag="acc", bufs=3,
                                         name=f"a{e}_{n}_{i}")
                            nc.vector.tensor_scalar_max(t_, z_ps, 0.0)
                            if e == 0:
                                nc.vector.tensor_scalar_mul(
                                    moe_parts[i][:, sl], t_, gcols[i][e])
                            else:
                                nc.vector.scalar_tensor_tensor(
                                    moe_parts[i][:, sl], t_, gcols[i][e],
                                    moe_parts[i][:, sl], ALU.mult, ALU.add)

                        # pairwise ReduceScatter as soon as chunk n done
                        if g > 1 and e == EL - 1:
                            for i in range(NB):
                                gdma.dma_start(
                                    ys[n][i * 128:(i + 1) * 128, :],
                                    moe_parts[i][:, sl])
                            nc.gpsimd.collective_compute(
                                "ReduceScatter",
                                bass.mybir.AluOpType.add,
                                replica_groups=groups,
                                ins=[ys[n].opt()],
                                outs=[rss[n].opt()],
                            )
                            gdma.dma_start(moe_r[:, sl], rss[n])

            # ---------------- stage 2: memory read + learned activation ------
            with tc.tile_pool(name="st2", bufs=1) as st2:
                if has_mrb:
                    mrb_row = st2.tile([1, M], f32, tag="mrb")
                    dma.dma_start(mrb_row, mrb_d[:])
                moeT_sb = st2.tile([128, KH * 128], f16, tag="moeT")
                exp_sb = st2.tile([128, M], f32, tag="exp")
                expT_sb = st2.tile([128, 16 * 128], f16, tag="expT")
                s_sb = st2.tile([128, H], f32, tag="s")
                out_sb = st2.tile([128, H], f32, tag="o")
                srec = st2.tile([128, 1], f32, tag="srec")

                with tc.tile_pool(name="pt", bufs=1, space="PSUM") as pt:
                    with tc.tile_pool(name="plg", bufs=1, space="PSUM") as plg:
                        lg = [plg.tile([128, 512], f32, tag="lg", bufs=4,
                                       name=f"lg{n}") for n in range(4)]
                        for ch in range(NH):
                            tp = pt.tile([128, 512], f32, tag="tp", bufs=2,
                                         name=f"tpm{ch}")
                            for j in range(4):
                                hk = ch * 4 + j
                                nc.tensor.transpose(tp[:, j * 128:(j + 1) * 128],
                                                    moe_r[:, hk * 128:(hk + 1) * 128],
                                                    identity)
                            nc.vector.tensor_copy(
                                moeT_sb[:, ch * 512:(ch + 1) * 512], tp)
                            for j in range(4):
                                hk = ch * 4 + j
                                for n in range(4):
                                    nc.tensor.matmul(
                                        lg[n],
                                        moeT_sb[:, hk * 128:(hk + 1) * 128],
                                        mrw_tiles[hk][:, n * 512:(n + 1) * 512],
                                        start=(hk == 0),
                                        stop=(hk == KH - 1) and not has_mrb)
                        if has_mrb:
                            for n in range(4):
                                nc.tensor.matmul(lg[n], ones_row[0:1, 0:1],
                                                 mrb_row[0:1, n * 512:(n + 1) * 512],
                                                 start=False, stop=True)

                        nmx = []
                        for n in range(4):
                            t_ = st2.tile([128, 1], f32, tag=f"nmx{n}", name=f"nmx{n}")
                            nc.vector.reduce_max(t_, lg[n], axis=AX.X, negate=True)
                            nmx.append(t_)
                        t01 = st2.tile([128, 1], f32, tag="t01")
                        nc.vector.tensor_scalar_min(t01, nmx[0], nmx[1])
                        t23 = st2.tile([128, 1], f32, tag="t23")
                        nc.vector.tensor_scalar_min(t23, nmx[2], nmx[3])
                        ngm2 = st2.tile([128, 1], f32, tag="ngm2")
                        nc.vector.tensor_scalar_min(ngm2, t01, t23)
                        ses = []
                        for n in range(4):
                            se_ = st2.tile([128, 1], f32, tag=f"ses{n}", name=f"ses{n}")
                            nc.scalar.activation(exp_sb[:, n * 512:(n + 1) * 512],
                                                 lg[n], AF.Exp, bias=ngm2,
                                                 accum_out=se_)
                            ses.append(se_)
                        s01 = st2.tile([128, 1], f32, tag="s01")
                        nc.vector.tensor_tensor(s01, ses[0], ses[1], ALU.add)
                        s23 = st2.tile([128, 1], f32, tag="s23")
                        nc.vector.tensor_tensor(s23, ses[2], ses[3], ALU.add)
                        stot = st2.tile([128, 1], f32, tag="stot")
                        nc.vector.tensor_tensor(stot, s01, s23, ALU.add)
                        nc.vector.reciprocal(srec, stot)

                    for t in range(4):
                        tp = pt.tile([128, 512], f32, tag="tp", bufs=2, name=f"tpe{t}")
                        for j in range(4):
                            mk = t * 4 + j
                            nc.tensor.transpose(tp[:, j * 128:(j + 1) * 128],
                                                exp_sb[:, mk * 128:(mk + 1) * 128],
                                                identity)
                        nc.vector.tensor_copy(expT_sb[:, t * 512:(t + 1) * 512],
                                              tp)

                # read matmul + blended activation, pipelined in column
                # halves: while half h runs its activation branches, half h+1
                # accumulates its read matmuls on the PE.
                # Mish is synthesized without softplus/ln:
                #   tanh(softplus(s)) == 1 - 2/((1+e^s)^2 + 1)  (s clamped at
                #   20, where the expression saturates to 1 in f32).
                with tc.tile_pool(name="prd", bufs=1, space="PSUM") as prd, \
                     tc.tile_pool(name="pac", bufs=1, space="PSUM") as pac, \
                     tc.tile_pool(name="brp", bufs=1) as brp:
                    HH = H // 2
                    n_groups = 7

                    for h in range(2):
                        hs = slice(h * HH, (h + 1) * HH)
                        rd = [prd.tile([128, 512], f32, tag="rd", bufs=4,
                                       name=f"rd{h}_{n}") for n in range(2)]
                        for mk in range(16):
                            for n in range(2):
                                nc.tensor.matmul(
                                    rd[n],
                                    expT_sb[:, mk * 128:(mk + 1) * 128],
                                    mem_tiles[mk][:, h * HH + n * 512:
                                                  h * HH + (n + 1) * 512],
                                    start=(mk == 0), stop=(mk == 15))
                        # s = moe + read_vec/sum (deferred normalization);
                        # columns >= h1 have moe == 0 by mask structure
                        for n in range(2):
                            sl = slice(h * HH + n * 512, h * HH + (n + 1) * 512)
                            if h * HH + n * 512 < h1:
                                nc.vector.scalar_tensor_tensor(
                                    s_sb[:, sl], rd[n], srec, moe_r[:, sl],
                                    ALU.mult, ALU.add)
                            else:
                                nc.vector.tensor_scalar_mul(s_sb[:, sl], rd[n],
                                                            srec)

                        s_h = s_sb[:, hs]
                        acc = [pac.tile([128, 512], f32, tag="acc", bufs=4,
                                        name=f"acc{h}_{n}") for n in range(2)]
                        gi = [0]

                        def acc_branch(br_tile, ci):
                            diag = brp.tile([128, 128], f32r, tag="d", bufs=2,
                                            name=f"d{h}_{gi[0]}")
                            nc.vector.tensor_scalar_mul(diag, identity,
                                                        coeffs_bc[:, ci:ci + 1])
                            for n in range(2):
                                nc.tensor.matmul(acc[n], diag,
                                                 br_tile[:, n * 512:(n + 1) * 512],
                                                 start=(gi[0] == 0),
                                                 stop=(gi[0] == n_groups - 1))
                            gi[0] += 1

                        # --- nl_exp table phase ---
                        relu_br = brp.tile([128, HH], f32r, tag="relu",
                                           bufs=2, name=f"rl{h}")
                        nc.scalar.activation(relu_br, s_h, AF.Relu)
                        acc_branch(relu_br, 5)
                        # exp(min(s,0)); the -1 of expm1 is folded into the
                        # final subtraction of c_em below
                        mn = brp.tile([128, HH], f32, tag="sc1", bufs=2,
                                      name=f"mn{h}")
                        nc.vector.tensor_scalar_min(mn, s_h, 0.0)
                        em_br = brp.tile([128, HH], f32r, tag="b", bufs=2,
                                         name=f"em{h}")
                        nc.scalar.activation(em_br, mn, AF.Exp)
                        acc_branch(em_br, 6)
                        # mish: v = e^min(s,20); tanh_sp = 1 - 2/((1+v)^2+1)
                        mn20 = brp.tile([128, HH], f32, tag="sc2", bufs=2,
                                        name=f"m20{h}")
                        nc.vector.tensor_scalar_min(mn20, s_h, 20.0)
                        v_br = brp.tile([128, HH], f32, tag="sc1", bufs=2,
                                        name=f"v{h}")
                        nc.scalar.activation(v_br, mn20, AF.Exp)
                        vp1 = brp.tile([128, HH], f32, tag="sc2", bufs=2,
                                       name=f"vp{h}")
                        nc.vector.tensor_scalar_add(vp1, v_br, 1.0)
                        w_t = brp.tile([128, HH], f32, tag="sc1", bufs=2,
                                       name=f"w{h}")
                        nc.vector.tensor_tensor(w_t, vp1, vp1, ALU.mult)
                        wp1 = brp.tile([128, HH], f32, tag="sc2", bufs=2,
                                       name=f"wp{h}")
                        nc.vector.tensor_scalar_add(wp1, w_t, 1.0)
                        r_t = brp.tile([128, HH], f32, tag="sc1", bufs=2,
                                       name=f"r{h}")
                        nc.vector.reciprocal(r_t, wp1)
                        tsp = brp.tile([128, HH], f32, tag="sc2", bufs=2,
                                       name=f"t{h}")
                        nc.vector.tensor_scalar(tsp, r_t, -2.0, 1.0,
                                                ALU.mult, ALU.add)
                        mish_br = brp.tile([128, HH], f32r, tag="b", bufs=2,
                                           name=f"mi{h}")
                        nc.vector.tensor_tensor(mish_br, tsp, s_h, ALU.mult)
                        acc_branch(mish_br, 4)
                        # --- sigmoid table phase ---
                        sg_br = brp.tile([128, HH], f32r, tag="b", bufs=2,
                                         name=f"sg{h}")
                        nc.scalar.activation(sg_br, s_h, AF.Sigmoid)
                        acc_branch(sg_br, 0)
                        th_br = brp.tile([128, HH], f32r, tag="b", bufs=2,
                                         name=f"th{h}")
                        nc.scalar.activation(th_br, s_h, AF.Tanh)
                        acc_branch(th_br, 1)
                        # silu = s * sigmoid(s), on the vector engine
                        sl_br = brp.tile([128, HH], f32r, tag="b", bufs=2,
                                         name=f"sl{h}")
                        nc.vector.tensor_tensor(sl_br, sg_br.bitcast(f32), s_h,
                                                ALU.mult)
                        acc_branch(sl_br, 2)
                        # --- gelu table phase ---
                        gl_br = brp.tile([128, HH], f32r, tag="b", bufs=2,
                                         name=f"gl{h}")
                        nc.scalar.activation(gl_br, s_h, AF.Gelu)
                        acc_branch(gl_br, 3)
                        assert gi[0] == n_groups
                        for n in range(2):
                            sl = slice(h * HH + n * 512, h * HH + (n + 1) * 512)
                            nc.vector.tensor_scalar_sub(out_sb[:, sl], acc[n],
                                                        coeffs_bc[:, 6:7])
                        dma.dma_start(out_d[:, hs], out_sb[:, hs])
    nc.finalize()
    return nc


def _get_nc(key=None):
    if key is None:
        key = _LAST_KEY
    if key not in _CACHED_NC:
        _CACHED_NC[key] = _build_program(*key)
    return _CACHED_NC[key]


def _r12(a):
    """Round fp32 to the fp32r grid (11 explicit mantissa bits, RNE)."""
    u = np.ascontiguousarray(a).view(np.uint32)
    u = (u + np.uint32(0x7FF) + ((u >> np.uint32(12)) & np.uint32(1))) \
        & np.uint32(0xFFFFF000)
    return u.view(np.float32)


def kernel(**inputs):
    import os
    from concourse.bass_utils import run_bass_kernel_spmd

    f = lambda a: np.ascontiguousarray(np.asarray(a, dtype=np.float32))
    x = f(inputs["x"])
    gate_w = f(inputs["gate_w"])
    expert_w = f(inputs["expert_w"])
    expert_b = f(inputs["expert_b"])
    conn_w1 = f(inputs["conn_w1"])
    conn_b1 = f(inputs["conn_b1"])
    conn_w2 = f(inputs["conn_w2"])
    conn_b2 = f(inputs["conn_b2"])
    neuron_avg = f(inputs["neuron_avg"])
    neuron_mask = f(inputs["neuron_mask"])
    mem_read_w = f(inputs["mem_read_w"])
    mem_read_b = f(inputs["mem_read_b"])
    memory = f(inputs["memory"])
    act_w = f(inputs["act_w"]).reshape(-1)

    g = int(os.environ.get("MOE_G", "1"))
    EL = E // g
    NB = g

    # host prep: softmax blend weights -> 7 branch coefficients
    p = np.exp(act_w - act_w.max())
    p = p / p.sum()
    coef = np.array([[p[0], p[2], p[4], p[5], p[7],
                      p[1] + p[3] + p[6] * SELU_SCALE,
                      p[1] + p[6] * SELU_SCALE * SELU_ALPHA, 0.0]], np.float32)

    # host conn MLP (batch-independent) -> cmask folded into expert weights
    h1v = np.einsum('eh,ehk->ek', neuron_avg, conn_w1) + conn_b1
    h1v = np.maximum(h1v, 0.0, dtype=np.float32)
    cl = np.einsum('ek,ekh->eh', h1v, conn_w2) + conn_b2
    conn = (1.0 / (1.0 + np.exp(-cl))).astype(np.float32)
    cmask = conn * neuron_mask                                   # [E, H]

    # stage-1 live width: columns past the last nonzero mask column are
    # structurally zero in moe_out, so the program skips them entirely
    nz = np.nonzero(neuron_mask.any(axis=0))[0]
    h1 = int(nz[-1]) + 1 if nz.size else 512
    h1 = min(H, max(512, -(-h1 // 512) * 512))

    wp = (expert_w[:, :, :h1] * cmask[:, None, :h1]).astype(np.float16)
    bp = (expert_b[:, :h1] * cmask[:, :h1]).astype(np.float16)
    has_eb = bool(np.any(bp))
    has_mrb = bool(np.any(mem_read_b))

    xT = np.ascontiguousarray(x.T).astype(np.float16)            # [D, B]
    xT_blk = xT.reshape(8, 128, B).transpose(1, 0, 2)            # [128, 8, B]
    mrw_bf = mem_read_w[:h1].reshape(h1 // 128, 128, M).astype(np.float16)
    mem_bf = memory.reshape(16, 128, H).astype(np.float16)
    mrb = np.ascontiguousarray(mem_read_b.reshape(1, M))

    in_maps = []
    for c in range(NCORES):
        if g > 1:
            bg, eg = c >> 1, c & 1
        else:
            bg, eg = c, 0
        gwr = np.roll(gate_w, -eg * EL, axis=1).astype(np.float16)
        ew_c = wp[eg * EL:(eg + 1) * EL]         # [EL, D, h1]
        m = {
            "xT": np.ascontiguousarray(
                xT_blk[:, :, bg * NB * 128:(bg + 1) * NB * 128]),
            "gw": np.ascontiguousarray(gwr.reshape(8, 128, E).transpose(1, 0, 2)),
            "ew": np.ascontiguousarray(
                ew_c.reshape(EL, 8, 128, h1).transpose(0, 2, 1, 3)),
            "mrw": mrw_bf,
            "mem": mem_bf,
            "coef": coef,
        }
        if has_eb:
            m["eb"] = np.ascontiguousarray(
                bp[eg * EL:(eg + 1) * EL].reshape(EL, 1, h1))
        if has_mrb:
            m["mrb"] = mrb
        in_maps.append(m)

    key = (h1, g, has_eb, has_mrb)
    global _LAST_IN_MAPS, _LAST_KEY
    _LAST_IN_MAPS = in_maps
    _LAST_KEY = key
    nc = _get_nc(key)
    results = run_bass_kernel_spmd(nc, in_maps, list(range(NCORES))).results
    out = np.concatenate(
        [np.asarray(results[c]["out"], dtype=np.float32) for c in range(NCORES)],
        axis=0)
    return out
